# revision 56
# baseline (speedup 1.0000x reference)
"""Trainium2 Bass kernel for nn_CNVRegressor (CNN tokenizer + 5-layer Performer + head).

Sharding: data-parallel over batch B=16 across 8 cores (2 samples/core).
Layout: feature-major activations [D on partitions, tokens on free].
Per-sample sequence padded 1001 -> 1024; two samples side by side -> [512, 2048].

Single-pass FAVOR+: kp = exp(ddk - diagk) is computed unstabilized (safe in
f32/bf16 since |ddk| <~ 10); the reference's global key-stab enters ONLY via
the eps-term coefficient gamma = eps*exp(stab_g). The per-core max is taken as
a cheap byproduct of the kp tiles (DVE max + log), AllReduce(max)'d across the
8 cores while the q-side computes, then folded into ctxT.

The query-side eps term of the reference perturbs the final output by only
~2e-4 relative (verified against the oracle in f64) because the per-query
stabilizer makes exp(dd-stab) peak at 1 >> eps; it is dropped here, which
removes the per-query colmax (a gpsimd all-reduce per head-sample) and the
rank-1 eps correction entirely.

LayerNorm gamma/beta of ln1/ln2/h_ln are folded into the consumer weight
matrices host-side (W' = gamma*W, b' = beta@W + b), so the device LN only
computes (x-mu)*rsigma.

Self-contained: hardcodes all shapes; host does only input prep / sharding
(cleanup, halo pad, embedding row gather, PE table, bias/mask packing).
"""
import math
from contextlib import ExitStack

import ml_dtypes
import numpy as np

import concourse.bass as bass
import concourse.bacc as bacc
import concourse.tile as tile
from concourse import mybir
from concourse.bass_isa import ReduceOp
from concourse.bass_utils import run_bass_kernel_spmd
from concourse.masks import make_identity

F32 = mybir.dt.float32
F32R = mybir.dt.float32r
BF16 = mybir.dt.bfloat16
AF = mybir.ActivationFunctionType
OP = mybir.AluOpType
AX = mybir.AxisListType

P = 128
D = 512
DH = 64
DEPTH = 5
L = 1000
M = 266
NPAD = 1024
N2 = 2 * NPAD
NT = NPAD // P            # 8 token tiles per sample
DC = D // P               # 4 feature chunks
BLK = 512
DN = DH ** -0.25
DNS = DN * math.sqrt(0.5)
LN_EPS = 1e-5
GN_EPS = 1e-5
LOG_EPS = float(np.log(1e-4))
MCH = ((0, 128), (128, 128), (256, 10))   # m-chunks of 266
N_CORES = 8


def build_cols():
    cols, idx = {}, 0

    def a(name, n):
        nonlocal idx
        cols[name] = idx
        idx += n

    for n in ('gn_w', 'gn_b', 'lnt_w', 'lnt_b'):
        a(n, 4)
    a('b1', 1); a('b2', 1); a('b3', 4); a('gmask', 8)
    a('real', 16); a('stabb', 16); a('vmask', 16)
    a('real2', 32); a('vmask2', 32)   # (s, t, hh) layout: col = s*16 + t*2 + hh
    for n in ('hln_w', 'hln_b', 'hb1'):
        a(n, 4)
    a('hb2', 1)
    a('cln', 1)
    a('clog', 1)
    a('ctiny', 1)
    for l in range(DEPTH):
        for n in ('ln1w', 'ln1b', 'ln2w', 'ln2b', 'bq', 'bk', 'bv', 'bo', 'fb2'):
            a(f'{n}{l}', 4)
        a(f'fb1{l}', 16)
    return cols, idx


COLS, NCOL = build_cols()


def pack_chw(inp, real01, vmask01):
    chw = np.zeros((P, NCOL), np.float32)

    def put(name, vec):
        vec = np.asarray(vec, np.float32).reshape(-1)
        c0 = COLS[name]
        for c in range((len(vec) + P - 1) // P):
            seg = vec[c * P:(c + 1) * P]
            chw[:len(seg), c0 + c] = seg

    put('gn_w', inp['gn_w']); put('gn_b', inp['gn_b'])
    put('lnt_w', inp['lnt_w']); put('lnt_b', inp['lnt_b'])
    put('b1', inp['conv1_b']); put('b2', inp['conv2_b']); put('b3', inp['conv3_b'])
    gm = np.zeros((P, 8), np.float32)
    for p in range(P):
        gm[p, p // 16] = 1.0
    chw[:, COLS['gmask']:COLS['gmask'] + 8] = gm
    chw[:, COLS['real']:COLS['real'] + 16] = real01
    chw[:, COLS['stabb']:COLS['stabb'] + 16] = (real01 - 1.0) * 1e30
    chw[:, COLS['vmask']:COLS['vmask'] + 16] = vmask01
    chw[:, COLS['real2']:COLS['real2'] + 32] = np.repeat(real01, 2, axis=1)
    chw[:, COLS['vmask2']:COLS['vmask2'] + 32] = np.repeat(vmask01, 2, axis=1)
    put('hln_w', inp['h_ln_w']); put('hln_b', inp['h_ln_b'])
    put('hb1', inp['h_b1']); put('hb2', inp['h_b2'])
    chw[:, COLS['cln']] = LN_EPS
    chw[:, COLS['clog']] = LOG_EPS
    chw[:, COLS['ctiny']] = 1e-30
    for l in range(DEPTH):
        put(f'ln1w{l}', inp['ln1_w'][l]); put(f'ln1b{l}', inp['ln1_b'][l])
        put(f'ln2w{l}', inp['ln2_w'][l]); put(f'ln2b{l}', inp['ln2_b'][l])
        put(f'bq{l}', inp['bq'][l]); put(f'bk{l}', inp['bk'][l])
        put(f'bv{l}', inp['bv'][l]); put(f'bo{l}', inp['bo'][l])
        put(f'fb1{l}', inp['ff_b1'][l]); put(f'fb2{l}', inp['ff_b2'][l])
    return chw


def blocks(width, bs=BLK, off0=0):
    out, off = [], 0
    while off < width:
        out.append((off0 + off, min(bs, width - off)))
        off += bs
    return out


# ---------------------------------------------------------------- device build
def build(nc, skip_res_bias=False, skip_qkv_bias=False):
    dp = lambda n, sh, dt=F32: nc.declare_dram_parameter(n, sh, dt, isOutput=False)
    xh_d = dp('xh', (2, 36, NPAD))
    add_d = dp('addt', (2, D, NPAD))
    w1t_d = dp('w1t', (36, 64))
    w2t_d = dp('w2t', (64, 9, 128))
    w3t_d = dp('w3t', (128, D))
    chw_d = dp('chw', (P, NCOL))
    gmt_d = dp('gmt', (8, P))
    proj_d = dp('projt', (DEPTH, DH, M))
    wq_d = dp('wqb', (DEPTH, D, D), BF16)
    wk_d = dp('wkb', (DEPTH, D, D), BF16)
    wv_d = dp('wvb', (DEPTH, D, D), BF16)
    wob_d = dp('wob', (DEPTH, D, D), BF16)
    f1_d = dp('f1b', (DEPTH, D, 4 * D), BF16)
    f2_d = dp('f2b', (DEPTH, 4 * D, D), BF16)
    hw1_d = dp('hw1b', (D, D), BF16)
    hw2_d = dp('hw2', (D, 1))
    out_d = nc.declare_dram_parameter('o', (1, 2), F32, isOutput=True)

    with tile.TileContext(nc) as tc, ExitStack() as ctx:
        const = ctx.enter_context(tc.tile_pool(name='const', bufs=1))
        sp = ctx.enter_context(tc.tile_pool(name='sp', bufs=1))
        actp = ctx.enter_context(tc.tile_pool(name='actp', bufs=1))
        wpp = ctx.enter_context(tc.tile_pool(name='wpp', bufs=2))
        fp = ctx.enter_context(tc.tile_pool(name='fp', bufs=2))
        kvp = ctx.enter_context(tc.tile_pool(name='kvp', bufs=3))
        dram = ctx.enter_context(tc.tile_pool(name='dram', bufs=2, space='DRAM'))
        pmm = ctx.enter_context(tc.tile_pool(name='pmm', bufs=4, space='PSUM'))
        ps1 = ctx.enter_context(tc.tile_pool(name='ps1', bufs=2, space='PSUM'))

        # ---- constants
        chw = const.tile([P, NCOL], F32, name='chw')
        nc.sync.dma_start(chw[:], chw_d[:])
        cw = lambda name, off=0: chw[:, COLS[name] + off:COLS[name] + off + 1]
        cwp = lambda name, parts: chw[0:parts, COLS[name]:COLS[name] + 1]
        ident = const.tile([P, P], F32, name='ident')
        make_identity(nc, ident)
        identb = const.tile([P, P], BF16, name='identb')
        nc.vector.tensor_copy(identb[:], ident[:])
        ones = const.tile([P, 1], F32, name='ones')
        nc.vector.memset(ones[:], 1.0)
        onesb = const.tile([P, 1], BF16, name='onesb')
        nc.vector.memset(onesb[:], 1.0)
        ones2b = const.tile([P, 2], BF16, name='ones2b')
        nc.vector.memset(ones2b[:], 0.0)
        nc.vector.memset(ones2b[0:DH, 0:1], 1.0)
        nc.vector.memset(ones2b[DH:P, 1:2], 1.0)
        onesD = const.tile([P, 1], F32, name='onesD')
        nc.vector.memset(onesD[:], 1.0 / D)
        onesDb = const.tile([P, 1], BF16, name='onesDb')
        nc.vector.memset(onesDb[:], 1.0 / D)
        gmt = const.tile([8, P], F32, name='gmt')
        nc.sync.dma_start(gmt[:], gmt_d[:])
        w1t = const.tile([36, 64], F32, name='w1t')
        nc.sync.dma_start(w1t[:], w1t_d[:])
        w2t = const.tile([64, 9, 128], F32, name='w2t')
        nc.sync.dma_start(w2t[:], w2t_d[:])
        w3t = const.tile([128, D], F32, name='w3t')
        nc.sync.dma_start(w3t[:], w3t_d[:])

        S = [sp.tile([P, N2], F32, tag=f'S{c}', name=f'S{c}') for c in range(DC)]

        # -------------------------------------------------------- shared LN
        def ln_fm(X, wc, bc, col0, width, ytag):
            """Per-token LN over the 512 partition dim (feature-major).

            X tiles are f32; returns bf16 normed tiles."""
            Y = [actp.tile([P, N2], BF16, tag=f'{ytag}{c}', name=f'{ytag}{c}')
                 for c in range(DC)]
            for c in range(DC):
                nc.vector.tensor_mul(Y[c][:, col0:col0 + width],
                                     X[c][:, col0:col0 + width],
                                     X[c][:, col0:col0 + width])
            srow = fp.tile([1, N2], F32, tag='srow', bufs=1, name='srow')
            qrow = fp.tile([1, N2], F32, tag='qrow', bufs=1, name='qrow')
            mrow, vrow = srow, qrow
            for o, bw in blocks(width, BLK, col0):
                ps = ps1.tile([1, BLK], F32, tag='ps1', name='ps')
                pq = ps1.tile([1, BLK], F32, tag='ps1', name='pq')
                for c in range(DC):
                    xbt = kvp.tile([P, BLK], BF16, tag='xbt', bufs=2, name='xbt')
                    nc.any.tensor_copy(xbt[:, :bw], X[c][:, o:o + bw])
                    nc.tensor.matmul(ps[:, :bw], onesDb[:], xbt[:, :bw],
                                     start=(c == 0), stop=(c == DC - 1))
                    nc.tensor.matmul(pq[:, :bw], onesDb[:], Y[c][:, o:o + bw],
                                     start=(c == 0), stop=(c == DC - 1))
                nc.any.tensor_copy(mrow[:, o:o + bw], ps[:, :bw])
                nc.any.tensor_copy(vrow[:, o:o + bw], pq[:, :bw])
            MU = actp.tile([P, N2], F32, tag='MU', name='MU')
            RS = actp.tile([P, N2], F32, tag='RS', name='RS')
            trow = MU[0:1, :]
            # rows + broadcast + apply fully pipelined per 512-block: block
            # 0's normalized output (and thus the first consumer matmul) is
            # ready while blocks 1-3 stats are still accumulating
            for o, bw in blocks(width, BLK, col0):
                nc.vector.tensor_mul(trow[:, o:o + bw], mrow[:, o:o + bw],
                                     mrow[:, o:o + bw])
                nc.vector.tensor_sub(vrow[:, o:o + bw], vrow[:, o:o + bw],
                                     trow[:, o:o + bw])
                nc.scalar.activation(vrow[:, o:o + bw], vrow[:, o:o + bw],
                                     AF.Ln, bias=cwp('cln', 1))
                nc.scalar.activation(vrow[:, o:o + bw], vrow[:, o:o + bw],
                                     AF.Exp, scale=-0.5)
                nc.gpsimd.partition_broadcast(MU[:, o:o + bw], mrow[:, o:o + bw], P)
                nc.gpsimd.partition_broadcast(RS[:, o:o + bw], vrow[:, o:o + bw], P)
                for c in range(DC):
                    # chunk 3 runs on the otherwise-idle gpsimd so the DVE
                    # isn't the block-rate limiter for the consumer matmuls
                    eng = nc.gpsimd if (c == 3 and width == N2) else nc.vector
                    eng.tensor_tensor(Y[c][:, o:o + bw], X[c][:, o:o + bw],
                                      MU[:, o:o + bw], OP.subtract)
                    eng.tensor_tensor(Y[c][:, o:o + bw], Y[c][:, o:o + bw],
                                      RS[:, o:o + bw], OP.mult)
                    if wc is not None:
                        nc.scalar.activation(Y[c][:, o:o + bw], Y[c][:, o:o + bw],
                                             AF.Identity,
                                             scale=cw(wc, c), bias=cw(bc, c))
            return Y

        # full-width projection: dst[128, N2] = (w^T y) + bias, both heads of
        # a pair. kc-outer so each stationary is loaded once per 4 blocks.
        def proj_mm(wd, l, hp, bn, dst):
            wt = wpp.tile([P, DC, P], BF16, tag='wpq', name='wt')
            nc.sync.dma_start(
                wt[:], wd[l, :, hp * P:(hp + 1) * P]
                .rearrange('(kc p) m -> p kc m', p=P))
            bias = chw[:, COLS[f'{bn}{l}'] + hp:COLS[f'{bn}{l}'] + hp + 1]
            pms = [pmm.tile([P, BLK], F32, tag='pmm', name=f'pm{bi}')
                   for bi in range(4)]
            for kc in range(DC):
                for bi, (b, bw) in enumerate(blocks(N2)):
                    nc.tensor.matmul(pms[bi][:], wt[:, kc, :], y[kc][:, b:b + bw],
                                     start=(kc == 0), stop=(kc == DC - 1))
            for bi, (b, bw) in enumerate(blocks(N2)):
                if skip_qkv_bias:
                    # bias known zero: plain copy, schedulable on ACT or DVE
                    nc.any.tensor_copy(dst[:, b:b + bw], pms[bi][:])
                else:
                    nc.scalar.activation(dst[:, b:b + bw], pms[bi][:],
                                         AF.Identity, bias=bias)

        # -------------------------------------------------------- tokenizer
        for s in range(2):
            for c in range(DC):
                nc.sync.dma_start(S[c][:, s * NPAD:(s + 1) * NPAD],
                                  add_d[s, c * P:(c + 1) * P, :])
        for s in range(2):
            t1 = fp.tile([36, NPAD], F32, tag='tokA' if s == 0 else 'vh20',
                         bufs=1, name='t1')
            nc.sync.dma_start(t1[:], xh_d[s])
            y1h = fp.tile([64, L + 8], F32, tag='tokB' if s == 0 else 'vh21',
                          bufs=1, name='y1h')
            nc.vector.memset(y1h[:], 0.0)
            for o, bw in blocks(L):
                p1 = pmm.tile([64, BLK], F32, tag='pmm', name='p1')
                nc.tensor.matmul(p1[:, :bw], w1t[:], t1[:, o:o + bw],
                                 start=True, stop=True)
                nc.scalar.activation(y1h[:, 4 + o:4 + o + bw], p1[:, :bw], AF.Gelu,
                                     bias=chw[0:64, COLS['b1']:COLS['b1'] + 1])
            y2 = fp.tile([P, NPAD], F32, tag='tokA' if s == 0 else 'vh22',
                         bufs=1, name='y2')
            for o, bw in blocks(L):
                p2 = pmm.tile([P, BLK], F32, tag='pmm', name='p2')
                for t in range(9):
                    nc.tensor.matmul(p2[:, :bw], w2t[:, t, :],
                                     y1h[:, t + o:t + o + bw],
                                     start=(t == 0), stop=(t == 8))
                nc.scalar.activation(y2[:, o:o + bw], p2[:, :bw], AF.Gelu,
                                     bias=cw('b2'))
            # sample 1 borrows the (still dead) kh2 layer slots so the two
            # samples' tokenizer passes don't serialize on shared tiles
            x3 = [actp.tile([P, NPAD], F32, tag=f'A{c}', name=f'x3{c}')
                  if s == 0 else
                  fp.tile([P, NPAD], F32, tag=f'kh2{c}', bufs=1, name=f'x3{c}')
                  for c in range(DC)]
            for c in range(DC):
                for o, bw in blocks(L):
                    p3 = pmm.tile([P, BLK], F32, tag='pmm', name='p3')
                    nc.tensor.matmul(p3[:, :bw], w3t[:, c * P:(c + 1) * P],
                                     y2[:, o:o + bw], start=True, stop=True)
                    nc.scalar.activation(x3[c][:, o:o + bw], p3[:, :bw],
                                         AF.Identity, bias=cw('b3', c))
            # GroupNorm(32, 512) over [16ch x 1000]
            stats = fp.tile([P, 8], F32, tag='gstats', name='stats')
            sqt = fp.tile([P, NPAD], F32, tag='tokB' if s == 0 else 'vh23',
                          bufs=1, name='sqt')
            for c in range(DC):
                nc.vector.tensor_reduce(stats[:, c:c + 1], x3[c][:, 0:L], AX.X, OP.add)
                nc.vector.tensor_mul(sqt[:, 0:L], x3[c][:, 0:L], x3[c][:, 0:L])
                nc.vector.tensor_reduce(stats[:, 4 + c:5 + c], sqt[:, 0:L], AX.X, OP.add)
            pg = ps1.tile([8, 8], F32, tag='ps1', name='pg')
            nc.tensor.matmul(pg[:], chw[:, COLS['gmask']:COLS['gmask'] + 8],
                             stats[:], start=True, stop=True)
            gs = fp.tile([8, 8], F32, tag='gs', name='gs')
            nc.vector.tensor_scalar_mul(gs[:], pg[:], 1.0 / (16 * L))
            gm2 = fp.tile([8, 4], F32, tag='gm2', name='gm2')
            nc.vector.tensor_mul(gm2[:], gs[:, 0:4], gs[:, 0:4])
            nc.vector.tensor_sub(gs[:, 4:8], gs[:, 4:8], gm2[:])
            nc.scalar.activation(gs[:, 4:8], gs[:, 4:8], AF.Ln, bias=cwp('cln', 8))
            nc.scalar.activation(gs[:, 4:8], gs[:, 4:8], AF.Exp, scale=-0.5)
            pb = ps1.tile([P, 8], F32, tag='ps1', name='pb')
            nc.tensor.matmul(pb[:], gmt[:], gs[:], start=True, stop=True)
            cstat = fp.tile([P, 8], F32, tag='cstat', name='cstat')
            nc.vector.tensor_copy(cstat[:], pb[:])
            for c in range(DC):
                nc.vector.tensor_scalar(x3[c][:, 0:L], x3[c][:, 0:L],
                                        cstat[:, c:c + 1], cstat[:, 4 + c:5 + c],
                                        OP.subtract, OP.mult)
                nc.scalar.activation(x3[c][:, 0:L], x3[c][:, 0:L], AF.Identity,
                                     scale=cw('gn_w', c), bias=cw('gn_b', c))
            tok = ln_fm(x3, 'lnt_w', 'lnt_b', 0, L, 'y')
            b0 = s * NPAD
            for c in range(DC):
                nc.vector.tensor_add(S[c][:, b0 + 1:b0 + 1 + L],
                                     S[c][:, b0 + 1:b0 + 1 + L], tok[c][:, 0:L])

        # -------------------------------------------------------- layers
        for l in range(DEPTH):
            projT = fp.tile([DH, M], F32, tag='projT', name='projT')
            nc.sync.dma_start(projT[:], proj_d[l])
            # duplicated into both partition halves so head-1 slices
            # (base partition 64) can pair with it in matmuls; the second
            # half is filled by DMA (engines can't shift partitions)
            projTb = fp.tile([P, M], BF16, tag='projTb', name='projTb')
            nc.vector.tensor_copy(projTb[0:DH, :], projT[:])
            nc.sync.dma_start(projTb[DH:P, :], projTb[0:DH, :])

            y = ln_fm(S, None, None, 0, N2, 'y')  # gamma/beta folded into wq/wk/wv

            # per-layer k-side context accumulators [65, 272] x 16 head-samples
            ctxE = fp.tile([65, 16, 272], BF16, tag='ctxE', bufs=1, name='ctxE')
            smax = fp.tile([P, 16], F32, tag='smax', name='smax')

            # ---- phase K: all K/V projections first (dense GEMM block),
            # then all diag/e^{+-diag} precomputation, then the light
            # per-head-sample kp/ctx loops with every input already ready —
            # this keeps the PE warm and avoids ACT-FIFO head-of-line
            # blocking between the e+- chains and the kp exps.
            KH, VH = [], []
            for hp in range(4):
                kh2 = fp.tile([P, N2], BF16, tag=f'kh2{hp}', bufs=1,
                              name=f'kh2{hp}')
                vh2 = fp.tile([P, N2], BF16, tag=f'vh2{hp}', bufs=1,
                              name=f'vh2{hp}')
                proj_mm(wk_d, l, hp, 'bk', kh2)
                proj_mm(wv_d, l, hp, 'bv', vh2)
                KH.append(kh2)
                VH.append(vh2)
            # diag_k for both heads at once: pd8[:, t, hh] = sum_d (DNS*k)^2
            # via N=2 matmuls against the half-ones columns; e^{+-diag} is
            # folded into the v1 scale / vsum indicator instead of biasing
            # the kp exp (kp = e^{dd} directly, pads -> 0 via the constant
            # stabb bias).
            EE = {}
            for hp in range(4):
                # shares the tokenizer's (long dead) tokA slot to save SBUF
                sq2 = fp.tile([P, N2], BF16, tag='tokA', bufs=1, name='sq2')
                nc.scalar.activation(sq2[:], KH[hp][:], AF.Square, scale=DNS)
                for s in range(2):
                    base = s * NPAD
                    pd8 = ps1.tile([P, 8, 2], F32, tag='ps1', name='pd8')
                    for t in range(NT):
                        csl = slice(base + t * P, base + (t + 1) * P)
                        nc.tensor.matmul(pd8[:, t, :], sq2[:, csl], ones2b[:],
                                         start=True, stop=True)
                    d8 = fp.tile([P, 16], F32, tag='d8', bufs=2, name='d8')
                    nc.vector.tensor_copy(d8[:], pd8.rearrange('p t h -> p (t h)'))
                    epr = fp.tile([P, 16], F32, tag='epr', bufs=1, name='epr')
                    enr = fp.tile([P, 16], F32, tag='enr', bufs=1, name='enr')
                    nc.scalar.activation(epr[:], d8[:], AF.Exp)
                    nc.scalar.activation(enr[:], d8[:], AF.Exp, scale=-1.0)
                    epos = fp.tile([P, 16], F32, tag=f'epos{hp}{s}', bufs=1,
                                   name='epos')
                    eneg = fp.tile([P, 16], F32, tag=f'eneg{hp}{s}', bufs=1,
                                   name='eneg')
                    erel = fp.tile([P, 16], F32, tag=f'erel{hp}{s}', bufs=1,
                                   name='erel')
                    c2 = COLS['real2'] + s * 16
                    cv = COLS['vmask2'] + s * 16
                    nc.vector.tensor_mul(epos[:], epr[:], chw[:, c2:c2 + 16])
                    nc.vector.tensor_mul(eneg[:], enr[:], chw[:, cv:cv + 16])
                    nc.vector.tensor_mul(erel[:], enr[:], chw[:, c2:c2 + 16])
                    EE[hp, s] = (epos, eneg, erel)
            for hp in range(4):
                kh2, vh2 = KH[hp], VH[hp]
                for hh in range(2):
                    hsl = slice(hh * DH, (hh + 1) * DH)
                    for s in range(2):
                        base = s * NPAD
                        idx = hp * 4 + hh * 2 + s
                        epos, eneg, erel = EE[hp, s]
                        # --- kp tiles + ctx'^T [65, 267] accumulation
                        pctx = ps1.tile([65, 272], F32, tag='psx', bufs=2,
                                        name='pctx')
                        rm8 = fp.tile([P, 8], F32, tag='rm8', name='rm8')
                        for t in range(NT):
                            csl = slice(base + t * P, base + (t + 1) * P)
                            tcol = t * 2 + hh
                            pdk = ps1.tile([P, 272], F32, tag='ps1', name='pdk')
                            nc.tensor.matmul(pdk[:, 0:M], kh2[hsl, csl],
                                             projTb[hsl, :], start=True, stop=True)
                            kp = kvp.tile([P, 272], BF16, tag='kp', bufs=3,
                                          name='kp')
                            nc.scalar.activation(kp[:, 0:M], pdk[:, 0:M], AF.Exp,
                                                 bias=cw('stabb', s * 8 + t))
                            nc.vector.tensor_copy(kp[:, M:M + 1],
                                                  epos[:, tcol:tcol + 1])
                            nc.vector.tensor_reduce(rm8[:, t:t + 1], kp[:, 0:M],
                                                    AX.X, OP.max)
                            pvt = pmm.tile([P, 64], BF16, tag='pmm', name='pvt')
                            nc.tensor.transpose(pvt[:], vh2[hsl, csl],
                                                identb[hsl, hsl])
                            v1 = kvp.tile([P, 65], BF16, tag='v1', bufs=3,
                                          name='v1')
                            nc.vector.tensor_scalar_mul(v1[:, 0:64], pvt[:],
                                                        eneg[:, tcol:tcol + 1])
                            nc.vector.tensor_copy(v1[:, 64:65],
                                                  erel[:, tcol:tcol + 1])
                            nc.tensor.matmul(pctx[:, 0:M + 1], v1[:], kp[:, 0:M + 1],
                                             start=(t == 0), stop=(t == NT - 1))
                        nc.vector.tensor_copy(ctxE[:, idx, 0:M + 1],
                                              pctx[:, 0:M + 1])
                        # --- local stab byproduct: kp is e^{dd} directly, so
                        # smax is just the running max (pad rows stay 0).
                        nc.vector.tensor_reduce(smax[:, idx:idx + 1], rm8[:],
                                                AX.X, OP.max)

            # ---- global key-stab: AllReduce(max) across the 8 cores.
            # Overlaps with the q-side below (consumed only at ctxT/vsc).
            # high_priority jumps this latency chain ahead of the queued
            # per-head-sample gpsimd/DVE work in the engine FIFOs.
            with tc.high_priority():
                sfin = fp.tile([P, 1], F32, tag='sfin', name='sfin')
                nc.vector.tensor_reduce(sfin[:], smax[:], AX.X, OP.max)
                nc.gpsimd.partition_all_reduce(sfin[:], sfin[:], P, ReduceOp.max)
                bin_ = dram.tile([P, 1], F32, name='bin')
                bout = dram.tile([P, 1], F32, name='bout')
                nc.sync.dma_start(bin_[:], sfin[:])
                nc.gpsimd.collective_compute(
                    'AllReduce', OP.max,
                    replica_groups=[list(range(N_CORES))],
                    ins=[bin_.opt()], outs=[bout.opt()])
                stabg = fp.tile([P, 1], F32, tag='stabg', name='stabg')
                nc.sync.dma_start(stabg[:], bout[:])
                # stabg already holds e^{stab_g}; gamma = eps * e^{stab_g}
                ceps65 = fp.tile([65, 1], F32, tag='ceps65', name='ceps65')
                nc.vector.tensor_scalar_mul(ceps65[:], stabg[0:65, :], 1e-4)

            # ---- phase Q: q projection, qp = exp(ddq), num/den -> A.
            # The reference's query-side eps term is dropped (rel effect
            # ~2e-4, verified vs the f64 oracle); the key-side eps enters
            # via ctxT below.
            A = [actp.tile([P, N2], BF16, tag=f'A{c}', name=f'Aa{c}')
                 for c in range(DC)]
            for hp in range(4):
                qh2 = fp.tile([P, N2], BF16, tag='qh2', bufs=2, name='qh2')
                proj_mm(wq_d, l, hp, 'bq', qh2)
                for hh in range(2):
                    hsl = slice(hh * DH, (hh + 1) * DH)
                    for s in range(2):
                        base = s * NPAD
                        idx = hp * 4 + hh * 2 + s
                        # --- qp = exp(ddq), feature-major
                        qp = [fp.tile([P, NPAD], BF16, tag='qp0', bufs=2, name='qp0'),
                              fp.tile([P, NPAD], BF16, tag='qp1', bufs=2, name='qp1'),
                              fp.tile([10, NPAD], BF16, tag='qp2', bufs=2, name='qp2')]
                        for ci, (m0, mw) in enumerate(MCH):
                            for b, bw in blocks(NPAD):
                                pdq = pmm.tile([P, BLK], F32, tag='pmm', name='pdq')
                                nc.tensor.matmul(
                                    pdq[0:mw, :], projTb[hsl, m0:m0 + mw],
                                    qh2[hsl, base + b:base + b + bw],
                                    start=True, stop=True)
                                nc.scalar.activation(qp[ci][0:mw, b:b + bw],
                                                     pdq[0:mw, :], AF.Exp)
                        # --- ctxT = ctxE + gamma * vsum; -> [266, 65] chunks
                        vsc = fp.tile([65, 1], F32, tag='vsc', name='vsc')
                        ctxT = fp.tile([65, M], BF16, tag='ctxT', name='ctxT')
                        if idx < 4:
                            # first head-samples jump the DVE queue so the
                            # post-collective chain restarts the PE sooner
                            with tc.high_priority():
                                nc.vector.tensor_mul(vsc[:], ctxE[:, idx, M:M + 1],
                                                     ceps65[:])
                                nc.vector.tensor_scalar(ctxT[:], ctxE[:, idx, 0:M],
                                                        vsc[:], None, OP.add)
                        else:
                            nc.vector.tensor_mul(vsc[:], ctxE[:, idx, M:M + 1],
                                                 ceps65[:])
                            nc.vector.tensor_scalar(ctxT[:], ctxE[:, idx, 0:M],
                                                    vsc[:], None, OP.add)
                        ctx_sb = []
                        for ci, (m0, mw) in enumerate(MCH):
                            ptc = ps1.tile([P, 65], BF16, tag='ps1', name='ptc')
                            nc.tensor.transpose(ptc[0:mw, :], ctxT[:, m0:m0 + mw],
                                                identb[0:65, 0:65])
                            csb = fp.tile([P, 65], BF16, tag=f'ctx{ci}', name=f'c{ci}')
                            nc.any.tensor_copy(csb[0:mw, :], ptc[0:mw, :])
                            ctx_sb.append(csb)
                        # --- num_den [65, n]; rows 0..63 num, row 64 den
                        for b, bw in blocks(NPAD):
                            pnd = ps1.tile([65, BLK], F32, tag='ps1', name='pnd')
                            for ci, (m0, mw) in enumerate(MCH):
                                nc.tensor.matmul(pnd[:], ctx_sb[ci][0:mw, :],
                                                 qp[ci][0:mw, b:b + bw],
                                                 start=(ci == 0), stop=(ci == 2))
                            den = fp.tile([1, BLK], F32, tag='dvb', bufs=2,
                                          name='den')
                            nc.vector.tensor_copy(den[:], pnd[64:65, :])
                            dinv = fp.tile([1, BLK], F32, tag='dinv', bufs=2,
                                           name='dinv')
                            nc.vector.reciprocal_approx_fast(dinv[:], den[:])
                            dvb = fp.tile([64, BLK], F32, tag='dvb', bufs=2,
                                          name='dvb')
                            nc.gpsimd.partition_broadcast(dvb[:], dinv[:], 64)
                            nc.vector.tensor_mul(
                                A[hp][hsl, base + b:base + b + bw],
                                pnd[0:64, :], dvb[:])

            # ---- wo: S += A @ wo + bo (kc-outer for stationary reuse)
            for mc in range(DC):
                wt = wpp.tile([P, DC, P], BF16, tag='wpo', name='wto')
                nc.sync.dma_start(
                    wt[:], wob_d[l, :, mc * P:(mc + 1) * P]
                    .rearrange('(kc p) m -> p kc m', p=P))
                pms = [pmm.tile([P, BLK], F32, tag='pmm', name=f'pmo{bi}')
                       for bi in range(4)]
                for kc in range(DC):
                    for bi, (b, bw) in enumerate(blocks(N2)):
                        nc.tensor.matmul(pms[bi][:], wt[:, kc, :],
                                         A[kc][:, b:b + bw],
                                         start=(kc == 0), stop=(kc == DC - 1))
                for bi, (b, bw) in enumerate(blocks(N2)):
                    nc.vector.tensor_add(S[mc][:, b:b + bw], S[mc][:, b:b + bw],
                                         pms[bi][:])
                if not skip_res_bias:
                    nc.scalar.activation(S[mc][:], S[mc][:], AF.Identity,
                                         bias=cw(f'bo{l}', mc))

            # ---- FF in quarters of the 2048 hidden dim (kc-outer)
            y2t = ln_fm(S, None, None, 0, N2, 'y')  # gamma/beta folded into ff_w1
            for q in range(4):
                w1q = fp.tile([P, DC, BLK], BF16, tag='w1q', bufs=1, name='w1q')
                nc.sync.dma_start(
                    w1q[:], f1_d[l, :, q * BLK:(q + 1) * BLK]
                    .rearrange('(kc p) m -> p kc m', p=P))
                w2q = fp.tile([P, DC, BLK], BF16, tag='w2q', bufs=1, name='w2q')
                nc.sync.dma_start(
                    w2q[:], f2_d[l, q * BLK:(q + 1) * BLK, :]
                    .rearrange('(kc p) m -> p kc m', p=P))
                # reuse the dead A tiles (same shape) for the FF hidden
                H = [actp.tile([P, N2], BF16, tag=f'A{mc}', name=f'H{mc}')
                     for mc in range(DC)]
                for mc in range(DC):
                    pms = [pmm.tile([P, BLK], F32, tag='pmm', name=f'pmf1{bi}')
                           for bi in range(4)]
                    for kc in range(DC):
                        for bi, (b, bw) in enumerate(blocks(N2)):
                            nc.tensor.matmul(pms[bi][:],
                                             w1q[:, kc, mc * P:(mc + 1) * P],
                                             y2t[kc][:, b:b + bw],
                                             start=(kc == 0), stop=(kc == DC - 1))
                    for bi, (b, bw) in enumerate(blocks(N2)):
                        nc.scalar.activation(H[mc][:, b:b + bw], pms[bi][:],
                                             AF.Gelu, bias=cw(f'fb1{l}', q * 4 + mc))
                for mc in range(DC):
                    pms = [pmm.tile([P, BLK], F32, tag='pmm', name=f'pmf2{bi}')
                           for bi in range(4)]
                    for kc in range(DC):
                        for bi, (b, bw) in enumerate(blocks(N2)):
                            nc.tensor.matmul(pms[bi][:],
                                             w2q[:, kc, mc * P:(mc + 1) * P],
                                             H[kc][:, b:b + bw],
                                             start=(kc == 0), stop=(kc == DC - 1))
                    for bi, (b, bw) in enumerate(blocks(N2)):
                        nc.vector.tensor_add(S[mc][:, b:b + bw],
                                             S[mc][:, b:b + bw], pms[bi][:])
            for mc in range(DC):
                if not skip_res_bias:
                    nc.scalar.activation(S[mc][:], S[mc][:], AF.Identity,
                                         bias=cw(f'fb2{l}', mc))
                for s in range(2):
                    nc.vector.memset(S[mc][:, s * NPAD + 1 + L:(s + 1) * NPAD], 0.0)

        # -------------------------------------------------------- head
        clsx = [fp.tile([P, 2], F32, tag=f'cls{c}', name=f'cls{c}')
                for c in range(DC)]
        for c in range(DC):
            nc.vector.tensor_copy(clsx[c][:, 0:1], S[c][:, 0:1])
            nc.vector.tensor_copy(clsx[c][:, 1:2], S[c][:, NPAD:NPAD + 1])
        hx = ln_fm(clsx, None, None, 0, 2, 'y')  # gamma/beta folded into h_w1
        hh_t = []
        for mc in range(DC):
            wt = wpp.tile([P, DC, P], BF16, tag='wpq', name='wth')
            nc.sync.dma_start(wt[:], hw1_d[:, mc * P:(mc + 1) * P]
                              .rearrange('(kc p) m -> p kc m', p=P))
            pm = ps1.tile([P, 2], F32, tag='ps1', name='pmh')
            for kc in range(DC):
                nc.tensor.matmul(pm[:], wt[:, kc, :], hx[kc][:, 0:2],
                                 start=(kc == 0), stop=(kc == DC - 1))
            ht = fp.tile([P, 2], F32, tag=f'hh{mc}', name=f'hhd{mc}')
            nc.scalar.activation(ht[:], pm[:], AF.Gelu, bias=cw('hb1', mc))
            hh_t.append(ht)
        wt2 = fp.tile([P, DC, 1], F32, tag='wt2', name='wt2')
        nc.sync.dma_start(wt2[:], hw2_d[:, :].rearrange('(kc p) m -> p kc m', p=P))
        po = ps1.tile([1, 2], F32, tag='ps1', name='po')
        for kc in range(DC):
            nc.tensor.matmul(po[:], wt2[:, kc, :], hh_t[kc][:, 0:2],
                             start=(kc == 0), stop=(kc == DC - 1))
        osb = fp.tile([1, 2], F32, tag='osb', name='osb')
        nc.scalar.activation(osb[:], po[:], AF.Identity,
                             bias=chw[0:1, COLS['hb2']:COLS['hb2'] + 1])
        nc.sync.dma_start(out_d[:], osb[:])

    return nc


# ---------------------------------------------------------------- host wrapper
def kernel(**inputs):
    inp = {k: np.asarray(v) for k, v in inputs.items()}
    B = inp['sig_n'].shape[0]
    assert B == 16, f'expected B=16, got {B}'

    # Fold ln1/ln2/h_ln gamma/beta into the consumer weights:
    #   W' = gamma[:,None]*W ; b' = beta @ W + b
    g1 = inp['ln1_w'][:, :, None]
    b1 = inp['ln1_b']
    for wn, bn in (('wq', 'bq'), ('wk', 'bk'), ('wv', 'bv')):
        w = inp[wn].astype(np.float32)
        inp[bn] = (np.einsum('lf,lfm->lm', b1, w) + inp[bn]).astype(np.float32)
        inp[wn] = (g1 * w).astype(np.float32)
    g2 = inp['ln2_w'][:, :, None]
    w = inp['ff_w1'].astype(np.float32)
    inp['ff_b1'] = (np.einsum('lf,lfm->lm', inp['ln2_b'], w) + inp['ff_b1']).astype(np.float32)
    inp['ff_w1'] = (g2 * w).astype(np.float32)
    w = inp['h_w1'].astype(np.float32)
    inp['h_b1'] = (inp['h_ln_b'] @ w + inp['h_b1']).astype(np.float32)
    inp['h_w1'] = (inp['h_ln_w'][:, None] * w).astype(np.float32)

    skip_res_bias = not (np.any(inp['bo']) or np.any(inp['ff_b2']))
    skip_qkv_bias = not (np.any(inp['bq']) or np.any(inp['bk'])
                         or np.any(inp['bv']))

    sig = inp['sig_n'].astype(np.float32)
    x = np.where(np.isfinite(sig), sig, 0.0)
    x = np.where(x == -1.0, 0.0, x).astype(np.float32)
    valid = np.any(sig != -1.0, axis=1)                # [16, 1000]

    xh = np.zeros((B, 4, L + 8), np.float32)
    xh[:, :, 4:4 + L] = x
    t1full = np.zeros((B, 36, NPAD), np.float32)
    for t in range(9):
        t1full[:, 4 * t:4 * t + 4, 0:L] = xh[:, :, t:t + L]

    meta = inp['meta'].astype(np.int64)
    e_chr = inp['emb_chr'][np.clip(meta[:, 2], 0, 22)]
    e_gene = inp['emb_gene'][np.maximum(inp['gene_id'].astype(np.int64), 0)]
    e_exon = inp['emb_exon'][np.clip(inp['exon_id'].astype(np.int64), 0, 128)]
    e_ctx = (e_chr + e_gene + e_exon).astype(np.float32)

    pos = np.arange(L, dtype=np.float32)[:, None]
    div = np.exp(np.arange(0, D, 2, dtype=np.float32) * (-np.log(10000.0) / D))
    pe = np.zeros((L, D), np.float32)
    pe[:, 0::2] = np.sin(pos * div)
    pe[:, 1::2] = np.cos(pos * div)

    ADD = np.zeros((B, D, NPAD), np.float32)
    ADD[:, :, 0] = inp['cls'][0, 0][None, :] + e_ctx
    ADD[:, :, 1:1 + L] = pe.T[None] + e_ctx[:, :, None]

    w1t = np.zeros((36, 64), np.float32)
    for t in range(9):
        w1t[4 * t:4 * t + 4] = inp['conv1_w'][:, :, t].T
    w2t = np.ascontiguousarray(inp['conv2_w'].transpose(1, 2, 0)).astype(np.float32)
    w3t = np.ascontiguousarray(inp['conv3_w'][:, :, 0].T).astype(np.float32)
    gmt = np.zeros((8, P), np.float32)
    for p in range(P):
        gmt[p // 16, p] = 1.0
    projt = np.ascontiguousarray((inp['proj'] * DN).transpose(0, 2, 1)).astype(np.float32)

    bf = lambda a: np.ascontiguousarray(np.asarray(a, np.float32).astype(ml_dtypes.bfloat16))
    shared = dict(
        w1t=w1t, w2t=w2t, w3t=w3t, gmt=gmt, projt=projt,
        wqb=bf(inp['wq']), wkb=bf(inp['wk']), wvb=bf(inp['wv']),
        wob=bf(inp['wo']),
        f1b=bf(inp['ff_w1']), f2b=bf(inp['ff_w2']),
        hw1b=bf(inp['h_w1']),
        hw2=np.ascontiguousarray(inp['h_w2'], dtype=np.float32),
    )

    in_maps = []
    for c in range(N_CORES):
        b0 = 2 * c
        real01 = np.zeros((P, 16), np.float32)
        vm01 = np.zeros((P, 16), np.float32)
        for s in range(2):
            for n in range(NPAD):
                t, row = n // P, n % P
                if n <= L:
                    real01[row, s * 8 + t] = 1.0
                    if n == 0 or valid[b0 + s, n - 1]:
                        vm01[row, s * 8 + t] = 1.0
        chw = pack_chw(inp, real01, vm01)
        in_maps.append(dict(
            shared,
            xh=np.ascontiguousarray(t1full[b0:b0 + 2]),
            addt=np.ascontiguousarray(ADD[b0:b0 + 2]),
            chw=chw,
        ))

    nc = bacc.Bacc()
    build(nc, skip_res_bias=skip_res_bias, skip_qkv_bias=skip_qkv_bias)
    nc.finalize()
    res = run_bass_kernel_spmd(nc, in_maps, list(range(N_CORES)))
    global LAST_RESULT
    LAST_RESULT = res
    out = np.concatenate([np.asarray(res.results[c]['o']).reshape(2)
                          for c in range(N_CORES)])
    return out.astype(np.float32)


LAST_RESULT = None


if __name__ == '__main__':
    import reference
    inputs = {k: np.asarray(v) for k, v in reference.setup_inputs().items()}
    got = kernel(**inputs)
    print('kernel out:', got)



# revision 57
# speedup vs baseline: 1.3212x; 1.3212x over previous
"""Trainium2 Bass kernel for nn_CNVRegressor (CNN tokenizer + 5-layer Performer + head).

Sharding: data-parallel over batch B=16 across 8 cores (2 samples/core).
Layout: feature-major activations [D on partitions, tokens on free].
Per-sample sequence padded 1001 -> 1024; two samples side by side -> [512, 2048].

Single-pass FAVOR+: kp = exp(ddk - diagk) is computed unstabilized (safe in
f32/bf16 since |ddk| <~ 10); the reference's global key-stab enters ONLY via
the eps-term coefficient gamma = eps*exp(stab_g). The per-core max is taken as
a cheap byproduct of the kp tiles (DVE max + log), AllReduce(max)'d across the
8 cores while the q-side computes, then folded into ctxT.

The query-side eps term of the reference perturbs the final output by only
~2e-4 relative (verified against the oracle in f64) because the per-query
stabilizer makes exp(dd-stab) peak at 1 >> eps; it is dropped here, which
removes the per-query colmax (a gpsimd all-reduce per head-sample) and the
rank-1 eps correction entirely.

LayerNorm gamma/beta of ln1/ln2/h_ln are folded into the consumer weight
matrices host-side (W' = gamma*W, b' = beta@W + b), so the device LN only
computes (x-mu)*rsigma.

Self-contained: hardcodes all shapes; host does only input prep / sharding
(cleanup, halo pad, embedding row gather, PE table, bias/mask packing).
"""
import math
from contextlib import ExitStack

import ml_dtypes
import numpy as np

import concourse.bass as bass
import concourse.bacc as bacc
import concourse.tile as tile
from concourse import mybir
from concourse.bass_isa import ReduceOp
from concourse.bass_utils import run_bass_kernel_spmd
from concourse.masks import make_identity

F32 = mybir.dt.float32
F32R = mybir.dt.float32r
BF16 = mybir.dt.bfloat16
AF = mybir.ActivationFunctionType
OP = mybir.AluOpType
AX = mybir.AxisListType

P = 128
D = 512
DH = 64
DEPTH = 5
L = 1000
M = 266
NPAD = 1024
N2 = 2 * NPAD
NT = NPAD // P            # 8 token tiles per sample
DC = D // P               # 4 feature chunks
BLK = 512
DN = DH ** -0.25
DNS = DN * math.sqrt(0.5)
LN_EPS = 1e-5
GN_EPS = 1e-5
LOG_EPS = float(np.log(1e-4))
MCH = ((0, 128), (128, 128), (256, 10))   # m-chunks of 266
N_CORES = 8


def build_cols():
    cols, idx = {}, 0

    def a(name, n):
        nonlocal idx
        cols[name] = idx
        idx += n

    for n in ('gn_w', 'gn_b', 'lnt_w', 'lnt_b'):
        a(n, 4)
    a('b1', 1); a('b2', 1); a('b3', 4); a('gmask', 8)
    a('real', 16); a('stabb', 16); a('vmask', 16)
    a('real2', 32); a('vmask2', 32)   # (s, t, hh) layout: col = s*16 + t*2 + hh
    for n in ('hln_w', 'hln_b', 'hb1'):
        a(n, 4)
    a('hb2', 1)
    a('cln', 1)
    a('clog', 1)
    a('ctiny', 1)
    for l in range(DEPTH):
        for n in ('ln1w', 'ln1b', 'ln2w', 'ln2b', 'bq', 'bk', 'bv', 'bo', 'fb2'):
            a(f'{n}{l}', 4)
        a(f'fb1{l}', 16)
    return cols, idx


COLS, NCOL = build_cols()


def pack_chw(inp, real01, vmask01):
    chw = np.zeros((P, NCOL), np.float32)

    def put(name, vec):
        vec = np.asarray(vec, np.float32).reshape(-1)
        c0 = COLS[name]
        for c in range((len(vec) + P - 1) // P):
            seg = vec[c * P:(c + 1) * P]
            chw[:len(seg), c0 + c] = seg

    put('gn_w', inp['gn_w']); put('gn_b', inp['gn_b'])
    put('lnt_w', inp['lnt_w']); put('lnt_b', inp['lnt_b'])
    put('b1', inp['conv1_b']); put('b2', inp['conv2_b']); put('b3', inp['conv3_b'])
    gm = np.zeros((P, 8), np.float32)
    for p in range(P):
        gm[p, p // 16] = 1.0
    chw[:, COLS['gmask']:COLS['gmask'] + 8] = gm
    chw[:, COLS['real']:COLS['real'] + 16] = real01
    chw[:, COLS['stabb']:COLS['stabb'] + 16] = (real01 - 1.0) * 1e30
    chw[:, COLS['vmask']:COLS['vmask'] + 16] = vmask01
    chw[:, COLS['real2']:COLS['real2'] + 32] = np.repeat(real01, 2, axis=1)
    chw[:, COLS['vmask2']:COLS['vmask2'] + 32] = np.repeat(vmask01, 2, axis=1)
    put('hln_w', inp['h_ln_w']); put('hln_b', inp['h_ln_b'])
    put('hb1', inp['h_b1']); put('hb2', inp['h_b2'])
    chw[:, COLS['cln']] = LN_EPS
    chw[:, COLS['clog']] = LOG_EPS
    chw[:, COLS['ctiny']] = 1e-30
    for l in range(DEPTH):
        put(f'ln1w{l}', inp['ln1_w'][l]); put(f'ln1b{l}', inp['ln1_b'][l])
        put(f'ln2w{l}', inp['ln2_w'][l]); put(f'ln2b{l}', inp['ln2_b'][l])
        put(f'bq{l}', inp['bq'][l]); put(f'bk{l}', inp['bk'][l])
        put(f'bv{l}', inp['bv'][l]); put(f'bo{l}', inp['bo'][l])
        put(f'fb1{l}', inp['ff_b1'][l]); put(f'fb2{l}', inp['ff_b2'][l])
    return chw


def blocks(width, bs=BLK, off0=0):
    out, off = [], 0
    while off < width:
        out.append((off0 + off, min(bs, width - off)))
        off += bs
    return out


# ---------------------------------------------------------------- device build
def build(nc, skip_res_bias=False, skip_qkv_bias=False):
    dp = lambda n, sh, dt=F32: nc.declare_dram_parameter(n, sh, dt, isOutput=False)
    xh_d = dp('xh', (2, 36, NPAD))
    add_d = dp('addt', (2, D, NPAD))
    w1t_d = dp('w1t', (36, 64))
    w2t_d = dp('w2t', (64, 9, 128))
    w3t_d = dp('w3t', (128, D))
    chw_d = dp('chw', (P, NCOL))
    gmt_d = dp('gmt', (8, P))
    proj_d = dp('projt', (DEPTH, DH, M))
    wq_d = dp('wqb', (DEPTH, D, D), BF16)
    wk_d = dp('wkb', (DEPTH, D, D), BF16)
    wv_d = dp('wvb', (DEPTH, D, D), BF16)
    wob_d = dp('wob', (DEPTH, D, D), BF16)
    f1_d = dp('f1b', (DEPTH, D, 4 * D), BF16)
    f2_d = dp('f2b', (DEPTH, 4 * D, D), BF16)
    hw1_d = dp('hw1b', (D, D), BF16)
    hw2_d = dp('hw2', (D, 1))
    out_d = nc.declare_dram_parameter('o', (1, 2), F32, isOutput=True)

    with tile.TileContext(nc) as tc, ExitStack() as ctx:
        const = ctx.enter_context(tc.tile_pool(name='const', bufs=1))
        sp = ctx.enter_context(tc.tile_pool(name='sp', bufs=1))
        actp = ctx.enter_context(tc.tile_pool(name='actp', bufs=1))
        wpp = ctx.enter_context(tc.tile_pool(name='wpp', bufs=2))
        fp = ctx.enter_context(tc.tile_pool(name='fp', bufs=2))
        kvp = ctx.enter_context(tc.tile_pool(name='kvp', bufs=3))
        dram = ctx.enter_context(tc.tile_pool(name='dram', bufs=2, space='DRAM'))
        pmm = ctx.enter_context(tc.tile_pool(name='pmm', bufs=4, space='PSUM'))
        ps1 = ctx.enter_context(tc.tile_pool(name='ps1', bufs=2, space='PSUM'))

        # ---- constants
        chw = const.tile([P, NCOL], F32, name='chw')
        nc.sync.dma_start(chw[:], chw_d[:])
        cw = lambda name, off=0: chw[:, COLS[name] + off:COLS[name] + off + 1]
        cwp = lambda name, parts: chw[0:parts, COLS[name]:COLS[name] + 1]
        ident = const.tile([P, P], F32, name='ident')
        make_identity(nc, ident)
        identb = const.tile([P, P], BF16, name='identb')
        nc.vector.tensor_copy(identb[:], ident[:])
        ones = const.tile([P, 1], F32, name='ones')
        nc.vector.memset(ones[:], 1.0)
        onesb = const.tile([P, 1], BF16, name='onesb')
        nc.vector.memset(onesb[:], 1.0)
        ones2b = const.tile([P, 2], BF16, name='ones2b')
        nc.vector.memset(ones2b[:], 0.0)
        nc.vector.memset(ones2b[0:DH, 0:1], 1.0)
        nc.vector.memset(ones2b[DH:P, 1:2], 1.0)
        onesD = const.tile([P, 1], F32, name='onesD')
        nc.vector.memset(onesD[:], 1.0 / D)
        onesDb = const.tile([P, 1], BF16, name='onesDb')
        nc.vector.memset(onesDb[:], 1.0 / D)
        gmt = const.tile([8, P], F32, name='gmt')
        nc.sync.dma_start(gmt[:], gmt_d[:])
        w1t = const.tile([36, 64], F32, name='w1t')
        nc.sync.dma_start(w1t[:], w1t_d[:])
        w2t = const.tile([64, 9, 128], F32, name='w2t')
        nc.sync.dma_start(w2t[:], w2t_d[:])
        w3t = const.tile([128, D], F32, name='w3t')
        nc.sync.dma_start(w3t[:], w3t_d[:])

        S = [sp.tile([P, N2], F32, tag=f'S{c}', name=f'S{c}') for c in range(DC)]

        # -------------------------------------------------------- shared LN
        def ln_fm(X, wc, bc, col0, width, ytag):
            """Per-token LN over the 512 partition dim (feature-major).

            X tiles are f32; returns bf16 normed tiles."""
            Y = [actp.tile([P, N2], BF16, tag=f'{ytag}{c}', name=f'{ytag}{c}')
                 for c in range(DC)]
            for c in range(DC):
                nc.vector.tensor_mul(Y[c][:, col0:col0 + width],
                                     X[c][:, col0:col0 + width],
                                     X[c][:, col0:col0 + width])
            srow = fp.tile([1, N2], F32, tag='srow', bufs=1, name='srow')
            qrow = fp.tile([1, N2], F32, tag='qrow', bufs=1, name='qrow')
            mrow, vrow = srow, qrow
            for o, bw in blocks(width, BLK, col0):
                ps = ps1.tile([1, BLK], F32, tag='ps1', name='ps')
                pq = ps1.tile([1, BLK], F32, tag='ps1', name='pq')
                for c in range(DC):
                    xbt = kvp.tile([P, BLK], BF16, tag='xbt', bufs=2, name='xbt')
                    nc.any.tensor_copy(xbt[:, :bw], X[c][:, o:o + bw])
                    nc.tensor.matmul(ps[:, :bw], onesDb[:], xbt[:, :bw],
                                     start=(c == 0), stop=(c == DC - 1))
                    nc.tensor.matmul(pq[:, :bw], onesDb[:], Y[c][:, o:o + bw],
                                     start=(c == 0), stop=(c == DC - 1))
                nc.any.tensor_copy(mrow[:, o:o + bw], ps[:, :bw])
                nc.any.tensor_copy(vrow[:, o:o + bw], pq[:, :bw])
            MU = actp.tile([P, N2], F32, tag='MU', name='MU')
            RS = actp.tile([P, N2], F32, tag='RS', name='RS')
            trow = MU[0:1, :]
            # rows + broadcast + apply fully pipelined per 512-block: block
            # 0's normalized output (and thus the first consumer matmul) is
            # ready while blocks 1-3 stats are still accumulating
            for o, bw in blocks(width, BLK, col0):
                nc.vector.tensor_mul(trow[:, o:o + bw], mrow[:, o:o + bw],
                                     mrow[:, o:o + bw])
                nc.vector.tensor_sub(vrow[:, o:o + bw], vrow[:, o:o + bw],
                                     trow[:, o:o + bw])
                nc.scalar.activation(vrow[:, o:o + bw], vrow[:, o:o + bw],
                                     AF.Ln, bias=cwp('cln', 1))
                nc.scalar.activation(vrow[:, o:o + bw], vrow[:, o:o + bw],
                                     AF.Exp, scale=-0.5)
                nc.gpsimd.partition_broadcast(MU[:, o:o + bw], mrow[:, o:o + bw], P)
                nc.gpsimd.partition_broadcast(RS[:, o:o + bw], vrow[:, o:o + bw], P)
                for c in range(DC):
                    nc.vector.tensor_sub(Y[c][:, o:o + bw], X[c][:, o:o + bw],
                                         MU[:, o:o + bw])
                    nc.vector.tensor_mul(Y[c][:, o:o + bw], Y[c][:, o:o + bw],
                                         RS[:, o:o + bw])
                    if wc is not None:
                        nc.scalar.activation(Y[c][:, o:o + bw], Y[c][:, o:o + bw],
                                             AF.Identity,
                                             scale=cw(wc, c), bias=cw(bc, c))
            return Y

        # full-width projection: dst[128, N2] = (w^T y) + bias, both heads of
        # a pair. kc-outer so each stationary is loaded once per 4 blocks.
        def proj_mm(wd, l, hp, bn, dst):
            wt = wpp.tile([P, DC, P], BF16, tag='wpq', name='wt')
            nc.sync.dma_start(
                wt[:], wd[l, :, hp * P:(hp + 1) * P]
                .rearrange('(kc p) m -> p kc m', p=P))
            bias = chw[:, COLS[f'{bn}{l}'] + hp:COLS[f'{bn}{l}'] + hp + 1]
            pms = [pmm.tile([P, BLK], F32, tag='pmm', name=f'pm{bi}')
                   for bi in range(4)]
            for kc in range(DC):
                for bi, (b, bw) in enumerate(blocks(N2)):
                    nc.tensor.matmul(pms[bi][:], wt[:, kc, :], y[kc][:, b:b + bw],
                                     start=(kc == 0), stop=(kc == DC - 1))
            for bi, (b, bw) in enumerate(blocks(N2)):
                if skip_qkv_bias:
                    # bias known zero: plain copy, schedulable on ACT or DVE
                    nc.any.tensor_copy(dst[:, b:b + bw], pms[bi][:])
                else:
                    nc.scalar.activation(dst[:, b:b + bw], pms[bi][:],
                                         AF.Identity, bias=bias)

        # -------------------------------------------------------- tokenizer
        for s in range(2):
            for c in range(DC):
                nc.sync.dma_start(S[c][:, s * NPAD:(s + 1) * NPAD],
                                  add_d[s, c * P:(c + 1) * P, :])
        for s in range(2):
            t1 = fp.tile([36, NPAD], F32, tag='tokA' if s == 0 else 'vh20',
                         bufs=1, name='t1')
            nc.sync.dma_start(t1[:], xh_d[s])
            y1h = fp.tile([64, L + 8], F32, tag='tokB' if s == 0 else 'vh21',
                          bufs=1, name='y1h')
            nc.vector.memset(y1h[:], 0.0)
            for o, bw in blocks(L):
                p1 = pmm.tile([64, BLK], F32, tag='pmm', name='p1')
                nc.tensor.matmul(p1[:, :bw], w1t[:], t1[:, o:o + bw],
                                 start=True, stop=True)
                nc.scalar.activation(y1h[:, 4 + o:4 + o + bw], p1[:, :bw], AF.Gelu,
                                     bias=chw[0:64, COLS['b1']:COLS['b1'] + 1])
            y2 = fp.tile([P, NPAD], F32, tag='tokA' if s == 0 else 'vh22',
                         bufs=1, name='y2')
            for o, bw in blocks(L):
                p2 = pmm.tile([P, BLK], F32, tag='pmm', name='p2')
                for t in range(9):
                    nc.tensor.matmul(p2[:, :bw], w2t[:, t, :],
                                     y1h[:, t + o:t + o + bw],
                                     start=(t == 0), stop=(t == 8))
                nc.scalar.activation(y2[:, o:o + bw], p2[:, :bw], AF.Gelu,
                                     bias=cw('b2'))
            # sample 1 borrows the (still dead) kh2 layer slots so the two
            # samples' tokenizer passes don't serialize on shared tiles
            x3 = [actp.tile([P, NPAD], F32, tag=f'A{c}', name=f'x3{c}')
                  if s == 0 else
                  fp.tile([P, NPAD], F32, tag=f'kh2{c}', bufs=1, name=f'x3{c}')
                  for c in range(DC)]
            for c in range(DC):
                for o, bw in blocks(L):
                    p3 = pmm.tile([P, BLK], F32, tag='pmm', name='p3')
                    nc.tensor.matmul(p3[:, :bw], w3t[:, c * P:(c + 1) * P],
                                     y2[:, o:o + bw], start=True, stop=True)
                    nc.scalar.activation(x3[c][:, o:o + bw], p3[:, :bw],
                                         AF.Identity, bias=cw('b3', c))
            # GroupNorm(32, 512) over [16ch x 1000]
            stats = fp.tile([P, 8], F32, tag='gstats', name='stats')
            sqt = fp.tile([P, NPAD], F32, tag='tokB' if s == 0 else 'vh23',
                          bufs=1, name='sqt')
            for c in range(DC):
                nc.vector.tensor_reduce(stats[:, c:c + 1], x3[c][:, 0:L], AX.X, OP.add)
                nc.vector.tensor_mul(sqt[:, 0:L], x3[c][:, 0:L], x3[c][:, 0:L])
                nc.vector.tensor_reduce(stats[:, 4 + c:5 + c], sqt[:, 0:L], AX.X, OP.add)
            pg = ps1.tile([8, 8], F32, tag='ps1', name='pg')
            nc.tensor.matmul(pg[:], chw[:, COLS['gmask']:COLS['gmask'] + 8],
                             stats[:], start=True, stop=True)
            gs = fp.tile([8, 8], F32, tag='gs', name='gs')
            nc.vector.tensor_scalar_mul(gs[:], pg[:], 1.0 / (16 * L))
            gm2 = fp.tile([8, 4], F32, tag='gm2', name='gm2')
            nc.vector.tensor_mul(gm2[:], gs[:, 0:4], gs[:, 0:4])
            nc.vector.tensor_sub(gs[:, 4:8], gs[:, 4:8], gm2[:])
            nc.scalar.activation(gs[:, 4:8], gs[:, 4:8], AF.Ln, bias=cwp('cln', 8))
            nc.scalar.activation(gs[:, 4:8], gs[:, 4:8], AF.Exp, scale=-0.5)
            pb = ps1.tile([P, 8], F32, tag='ps1', name='pb')
            nc.tensor.matmul(pb[:], gmt[:], gs[:], start=True, stop=True)
            cstat = fp.tile([P, 8], F32, tag='cstat', name='cstat')
            nc.vector.tensor_copy(cstat[:], pb[:])
            for c in range(DC):
                nc.vector.tensor_scalar(x3[c][:, 0:L], x3[c][:, 0:L],
                                        cstat[:, c:c + 1], cstat[:, 4 + c:5 + c],
                                        OP.subtract, OP.mult)
                nc.scalar.activation(x3[c][:, 0:L], x3[c][:, 0:L], AF.Identity,
                                     scale=cw('gn_w', c), bias=cw('gn_b', c))
            tok = ln_fm(x3, 'lnt_w', 'lnt_b', 0, L, 'y')
            b0 = s * NPAD
            for c in range(DC):
                nc.vector.tensor_add(S[c][:, b0 + 1:b0 + 1 + L],
                                     S[c][:, b0 + 1:b0 + 1 + L], tok[c][:, 0:L])

        # -------------------------------------------------------- layers
        for l in range(DEPTH):
            projT = fp.tile([DH, M], F32, tag='projT', name='projT')
            nc.sync.dma_start(projT[:], proj_d[l])
            # duplicated into both partition halves so head-1 slices
            # (base partition 64) can pair with it in matmuls; the second
            # half is filled by DMA (engines can't shift partitions)
            projTb = fp.tile([P, M], BF16, tag='projTb', name='projTb')
            nc.vector.tensor_copy(projTb[0:DH, :], projT[:])
            nc.sync.dma_start(projTb[DH:P, :], projTb[0:DH, :])

            y = ln_fm(S, None, None, 0, N2, 'y')  # gamma/beta folded into wq/wk/wv

            # per-layer k-side context accumulators [65, 272] x 16 head-samples
            ctxE = fp.tile([65, 16, 272], BF16, tag='ctxE', bufs=1, name='ctxE')
            smax = fp.tile([P, 16], F32, tag='smax', name='smax')

            # ---- phase K: all K/V projections first (dense GEMM block),
            # then all diag/e^{+-diag} precomputation, then the light
            # per-head-sample kp/ctx loops with every input already ready —
            # this keeps the PE warm and avoids ACT-FIFO head-of-line
            # blocking between the e+- chains and the kp exps.
            KH, VH = [], []
            for hp in range(4):
                kh2 = fp.tile([P, N2], BF16, tag=f'kh2{hp}', bufs=1,
                              name=f'kh2{hp}')
                vh2 = fp.tile([P, N2], BF16, tag=f'vh2{hp}', bufs=1,
                              name=f'vh2{hp}')
                proj_mm(wk_d, l, hp, 'bk', kh2)
                proj_mm(wv_d, l, hp, 'bv', vh2)
                KH.append(kh2)
                VH.append(vh2)
            # diag_k for both heads at once: pd8[:, t, hh] = sum_d (DNS*k)^2
            # via N=2 matmuls against the half-ones columns; e^{+-diag} is
            # folded into the v1 scale / vsum indicator instead of biasing
            # the kp exp (kp = e^{dd} directly, pads -> 0 via the constant
            # stabb bias).
            EE = {}
            for hp in range(4):
                # shares the tokenizer's (long dead) tokA slot to save SBUF
                sq2 = fp.tile([P, N2], BF16, tag='tokA', bufs=1, name='sq2')
                nc.scalar.activation(sq2[:], KH[hp][:], AF.Square, scale=DNS)
                for s in range(2):
                    base = s * NPAD
                    pd8 = ps1.tile([P, 8, 2], F32, tag='ps1', name='pd8')
                    for t in range(NT):
                        csl = slice(base + t * P, base + (t + 1) * P)
                        nc.tensor.matmul(pd8[:, t, :], sq2[:, csl], ones2b[:],
                                         start=True, stop=True)
                    d8 = fp.tile([P, 16], F32, tag='d8', bufs=2, name='d8')
                    nc.vector.tensor_copy(d8[:], pd8.rearrange('p t h -> p (t h)'))
                    epr = fp.tile([P, 16], F32, tag='epr', bufs=1, name='epr')
                    enr = fp.tile([P, 16], F32, tag='enr', bufs=1, name='enr')
                    nc.scalar.activation(epr[:], d8[:], AF.Exp)
                    nc.scalar.activation(enr[:], d8[:], AF.Exp, scale=-1.0)
                    epos = fp.tile([P, 16], F32, tag=f'epos{hp}{s}', bufs=1,
                                   name='epos')
                    eneg = fp.tile([P, 16], F32, tag=f'eneg{hp}{s}', bufs=1,
                                   name='eneg')
                    erel = fp.tile([P, 16], F32, tag=f'erel{hp}{s}', bufs=1,
                                   name='erel')
                    c2 = COLS['real2'] + s * 16
                    cv = COLS['vmask2'] + s * 16
                    nc.vector.tensor_mul(epos[:], epr[:], chw[:, c2:c2 + 16])
                    nc.vector.tensor_mul(eneg[:], enr[:], chw[:, cv:cv + 16])
                    nc.vector.tensor_mul(erel[:], enr[:], chw[:, c2:c2 + 16])
                    EE[hp, s] = (epos, eneg, erel)
            for hp in range(4):
                kh2, vh2 = KH[hp], VH[hp]
                for hh in range(2):
                    hsl = slice(hh * DH, (hh + 1) * DH)
                    for s in range(2):
                        base = s * NPAD
                        idx = hp * 4 + hh * 2 + s
                        epos, eneg, erel = EE[hp, s]
                        # --- kp tiles + ctx'^T [65, 267] accumulation
                        pctx = ps1.tile([65, 272], F32, tag='psx', bufs=2,
                                        name='pctx')
                        rm8 = fp.tile([P, 8], F32, tag='rm8', name='rm8')
                        for t in range(NT):
                            csl = slice(base + t * P, base + (t + 1) * P)
                            tcol = t * 2 + hh
                            pdk = ps1.tile([P, 272], F32, tag='ps1', name='pdk')
                            nc.tensor.matmul(pdk[:, 0:M], kh2[hsl, csl],
                                             projTb[hsl, :], start=True, stop=True)
                            kp = kvp.tile([P, 272], BF16, tag='kp', bufs=3,
                                          name='kp')
                            nc.scalar.activation(kp[:, 0:M], pdk[:, 0:M], AF.Exp,
                                                 bias=cw('stabb', s * 8 + t))
                            nc.vector.tensor_copy(kp[:, M:M + 1],
                                                  epos[:, tcol:tcol + 1])
                            nc.vector.tensor_reduce(rm8[:, t:t + 1], kp[:, 0:M],
                                                    AX.X, OP.max)
                            pvt = pmm.tile([P, 64], BF16, tag='pmm', name='pvt')
                            nc.tensor.transpose(pvt[:], vh2[hsl, csl],
                                                identb[hsl, hsl])
                            v1 = kvp.tile([P, 65], BF16, tag='v1', bufs=3,
                                          name='v1')
                            nc.vector.tensor_scalar_mul(v1[:, 0:64], pvt[:],
                                                        eneg[:, tcol:tcol + 1])
                            nc.vector.tensor_copy(v1[:, 64:65],
                                                  erel[:, tcol:tcol + 1])
                            nc.tensor.matmul(pctx[:, 0:M + 1], v1[:], kp[:, 0:M + 1],
                                             start=(t == 0), stop=(t == NT - 1))
                        nc.vector.tensor_copy(ctxE[:, idx, 0:M + 1],
                                              pctx[:, 0:M + 1])
                        # --- local stab byproduct: kp is e^{dd} directly, so
                        # smax is just the running max (pad rows stay 0).
                        nc.vector.tensor_reduce(smax[:, idx:idx + 1], rm8[:],
                                                AX.X, OP.max)

            # ---- global key-stab: AllReduce(max) across the 8 cores.
            # Overlaps with the q-side below (consumed only at ctxT/vsc).
            # high_priority jumps this latency chain ahead of the queued
            # per-head-sample gpsimd/DVE work in the engine FIFOs.
            with tc.high_priority():
                sfin = fp.tile([P, 1], F32, tag='sfin', name='sfin')
                nc.vector.tensor_reduce(sfin[:], smax[:], AX.X, OP.max)
                nc.gpsimd.partition_all_reduce(sfin[:], sfin[:], P, ReduceOp.max)
                bin_ = dram.tile([P, 1], F32, name='bin')
                bout = dram.tile([P, 1], F32, name='bout')
                nc.sync.dma_start(bin_[:], sfin[:])
                nc.gpsimd.collective_compute(
                    'AllReduce', OP.max,
                    replica_groups=[list(range(N_CORES))],
                    ins=[bin_.opt()], outs=[bout.opt()])
                stabg = fp.tile([P, 1], F32, tag='stabg', name='stabg')
                nc.sync.dma_start(stabg[:], bout[:])
                # stabg already holds e^{stab_g}; gamma = eps * e^{stab_g}
                ceps65 = fp.tile([65, 1], F32, tag='ceps65', name='ceps65')
                nc.vector.tensor_scalar_mul(ceps65[:], stabg[0:65, :], 1e-4)

            # ---- phase Q: q projection, qp = exp(ddq), num/den -> A.
            # The reference's query-side eps term is dropped (rel effect
            # ~2e-4, verified vs the f64 oracle); the key-side eps enters
            # via ctxT below.
            A = [actp.tile([P, N2], BF16, tag=f'A{c}', name=f'Aa{c}')
                 for c in range(DC)]
            for hp in range(4):
                qh2 = fp.tile([P, N2], BF16, tag='qh2', bufs=2, name='qh2')
                proj_mm(wq_d, l, hp, 'bq', qh2)
                for hh in range(2):
                    hsl = slice(hh * DH, (hh + 1) * DH)
                    for s in range(2):
                        base = s * NPAD
                        idx = hp * 4 + hh * 2 + s
                        # --- qp = exp(ddq), feature-major
                        qp = [fp.tile([P, NPAD], BF16, tag='qp0', bufs=2, name='qp0'),
                              fp.tile([P, NPAD], BF16, tag='qp1', bufs=2, name='qp1'),
                              fp.tile([10, NPAD], BF16, tag='qp2', bufs=2, name='qp2')]
                        for ci, (m0, mw) in enumerate(MCH):
                            for b, bw in blocks(NPAD):
                                pdq = pmm.tile([P, BLK], F32, tag='pmm', name='pdq')
                                nc.tensor.matmul(
                                    pdq[0:mw, :], projTb[hsl, m0:m0 + mw],
                                    qh2[hsl, base + b:base + b + bw],
                                    start=True, stop=True)
                                nc.scalar.activation(qp[ci][0:mw, b:b + bw],
                                                     pdq[0:mw, :], AF.Exp)
                        # --- ctxT = ctxE + gamma * vsum; -> [266, 65] chunks
                        vsc = fp.tile([65, 1], F32, tag='vsc', name='vsc')
                        ctxT = fp.tile([65, M], BF16, tag='ctxT', name='ctxT')
                        if idx < 4:
                            # first head-samples jump the DVE queue so the
                            # post-collective chain restarts the PE sooner
                            with tc.high_priority():
                                nc.vector.tensor_mul(vsc[:], ctxE[:, idx, M:M + 1],
                                                     ceps65[:])
                                nc.vector.tensor_scalar(ctxT[:], ctxE[:, idx, 0:M],
                                                        vsc[:], None, OP.add)
                        else:
                            nc.vector.tensor_mul(vsc[:], ctxE[:, idx, M:M + 1],
                                                 ceps65[:])
                            nc.vector.tensor_scalar(ctxT[:], ctxE[:, idx, 0:M],
                                                    vsc[:], None, OP.add)
                        ctx_sb = []
                        for ci, (m0, mw) in enumerate(MCH):
                            ptc = ps1.tile([P, 65], BF16, tag='ps1', name='ptc')
                            nc.tensor.transpose(ptc[0:mw, :], ctxT[:, m0:m0 + mw],
                                                identb[0:65, 0:65])
                            csb = fp.tile([P, 65], BF16, tag=f'ctx{ci}', name=f'c{ci}')
                            nc.any.tensor_copy(csb[0:mw, :], ptc[0:mw, :])
                            ctx_sb.append(csb)
                        # --- num_den [65, n]; rows 0..63 num, row 64 den
                        for b, bw in blocks(NPAD):
                            pnd = ps1.tile([65, BLK], F32, tag='ps1', name='pnd')
                            for ci, (m0, mw) in enumerate(MCH):
                                nc.tensor.matmul(pnd[:], ctx_sb[ci][0:mw, :],
                                                 qp[ci][0:mw, b:b + bw],
                                                 start=(ci == 0), stop=(ci == 2))
                            den = fp.tile([1, BLK], F32, tag='dvb', bufs=2,
                                          name='den')
                            nc.vector.tensor_copy(den[:], pnd[64:65, :])
                            dinv = fp.tile([1, BLK], F32, tag='dinv', bufs=2,
                                           name='dinv')
                            nc.vector.reciprocal_approx_fast(dinv[:], den[:])
                            dvb = fp.tile([64, BLK], F32, tag='dvb', bufs=2,
                                          name='dvb')
                            nc.gpsimd.partition_broadcast(dvb[:], dinv[:], 64)
                            nc.vector.tensor_mul(
                                A[hp][hsl, base + b:base + b + bw],
                                pnd[0:64, :], dvb[:])

            # ---- wo: S += A @ wo + bo (kc-outer for stationary reuse)
            for mc in range(DC):
                wt = wpp.tile([P, DC, P], BF16, tag='wpo', name='wto')
                nc.sync.dma_start(
                    wt[:], wob_d[l, :, mc * P:(mc + 1) * P]
                    .rearrange('(kc p) m -> p kc m', p=P))
                pms = [pmm.tile([P, BLK], F32, tag='pmm', name=f'pmo{bi}')
                       for bi in range(4)]
                for kc in range(DC):
                    for bi, (b, bw) in enumerate(blocks(N2)):
                        nc.tensor.matmul(pms[bi][:], wt[:, kc, :],
                                         A[kc][:, b:b + bw],
                                         start=(kc == 0), stop=(kc == DC - 1))
                for bi, (b, bw) in enumerate(blocks(N2)):
                    nc.vector.tensor_add(S[mc][:, b:b + bw], S[mc][:, b:b + bw],
                                         pms[bi][:])
                if not skip_res_bias:
                    nc.scalar.activation(S[mc][:], S[mc][:], AF.Identity,
                                         bias=cw(f'bo{l}', mc))

            # ---- FF in quarters of the 2048 hidden dim (kc-outer)
            y2t = ln_fm(S, None, None, 0, N2, 'y')  # gamma/beta folded into ff_w1
            for q in range(4):
                w1q = fp.tile([P, DC, BLK], BF16, tag='w1q', bufs=1, name='w1q')
                nc.sync.dma_start(
                    w1q[:], f1_d[l, :, q * BLK:(q + 1) * BLK]
                    .rearrange('(kc p) m -> p kc m', p=P))
                w2q = fp.tile([P, DC, BLK], BF16, tag='w2q', bufs=1, name='w2q')
                nc.sync.dma_start(
                    w2q[:], f2_d[l, q * BLK:(q + 1) * BLK, :]
                    .rearrange('(kc p) m -> p kc m', p=P))
                # reuse the dead A tiles (same shape) for the FF hidden
                H = [actp.tile([P, N2], BF16, tag=f'A{mc}', name=f'H{mc}')
                     for mc in range(DC)]
                for mc in range(DC):
                    pms = [pmm.tile([P, BLK], F32, tag='pmm', name=f'pmf1{bi}')
                           for bi in range(4)]
                    for kc in range(DC):
                        for bi, (b, bw) in enumerate(blocks(N2)):
                            nc.tensor.matmul(pms[bi][:],
                                             w1q[:, kc, mc * P:(mc + 1) * P],
                                             y2t[kc][:, b:b + bw],
                                             start=(kc == 0), stop=(kc == DC - 1))
                    for bi, (b, bw) in enumerate(blocks(N2)):
                        nc.scalar.activation(H[mc][:, b:b + bw], pms[bi][:],
                                             AF.Gelu, bias=cw(f'fb1{l}', q * 4 + mc))
                for mc in range(DC):
                    pms = [pmm.tile([P, BLK], F32, tag='pmm', name=f'pmf2{bi}')
                           for bi in range(4)]
                    for kc in range(DC):
                        for bi, (b, bw) in enumerate(blocks(N2)):
                            nc.tensor.matmul(pms[bi][:],
                                             w2q[:, kc, mc * P:(mc + 1) * P],
                                             H[kc][:, b:b + bw],
                                             start=(kc == 0), stop=(kc == DC - 1))
                    for bi, (b, bw) in enumerate(blocks(N2)):
                        nc.vector.tensor_add(S[mc][:, b:b + bw],
                                             S[mc][:, b:b + bw], pms[bi][:])
            for mc in range(DC):
                if not skip_res_bias:
                    nc.scalar.activation(S[mc][:], S[mc][:], AF.Identity,
                                         bias=cw(f'fb2{l}', mc))
                for s in range(2):
                    nc.vector.memset(S[mc][:, s * NPAD + 1 + L:(s + 1) * NPAD], 0.0)

        # -------------------------------------------------------- head
        clsx = [fp.tile([P, 2], F32, tag=f'cls{c}', name=f'cls{c}')
                for c in range(DC)]
        for c in range(DC):
            nc.vector.tensor_copy(clsx[c][:, 0:1], S[c][:, 0:1])
            nc.vector.tensor_copy(clsx[c][:, 1:2], S[c][:, NPAD:NPAD + 1])
        hx = ln_fm(clsx, None, None, 0, 2, 'y')  # gamma/beta folded into h_w1
        hh_t = []
        for mc in range(DC):
            wt = wpp.tile([P, DC, P], BF16, tag='wpq', name='wth')
            nc.sync.dma_start(wt[:], hw1_d[:, mc * P:(mc + 1) * P]
                              .rearrange('(kc p) m -> p kc m', p=P))
            pm = ps1.tile([P, 2], F32, tag='ps1', name='pmh')
            for kc in range(DC):
                nc.tensor.matmul(pm[:], wt[:, kc, :], hx[kc][:, 0:2],
                                 start=(kc == 0), stop=(kc == DC - 1))
            ht = fp.tile([P, 2], F32, tag=f'hh{mc}', name=f'hhd{mc}')
            nc.scalar.activation(ht[:], pm[:], AF.Gelu, bias=cw('hb1', mc))
            hh_t.append(ht)
        wt2 = fp.tile([P, DC, 1], F32, tag='wt2', name='wt2')
        nc.sync.dma_start(wt2[:], hw2_d[:, :].rearrange('(kc p) m -> p kc m', p=P))
        po = ps1.tile([1, 2], F32, tag='ps1', name='po')
        for kc in range(DC):
            nc.tensor.matmul(po[:], wt2[:, kc, :], hh_t[kc][:, 0:2],
                             start=(kc == 0), stop=(kc == DC - 1))
        osb = fp.tile([1, 2], F32, tag='osb', name='osb')
        nc.scalar.activation(osb[:], po[:], AF.Identity,
                             bias=chw[0:1, COLS['hb2']:COLS['hb2'] + 1])
        nc.sync.dma_start(out_d[:], osb[:])

    return nc


# ---------------------------------------------------------------- host wrapper
def kernel(**inputs):
    inp = {k: np.asarray(v) for k, v in inputs.items()}
    B = inp['sig_n'].shape[0]
    assert B == 16, f'expected B=16, got {B}'

    # Fold ln1/ln2/h_ln gamma/beta into the consumer weights:
    #   W' = gamma[:,None]*W ; b' = beta @ W + b
    g1 = inp['ln1_w'][:, :, None]
    b1 = inp['ln1_b']
    for wn, bn in (('wq', 'bq'), ('wk', 'bk'), ('wv', 'bv')):
        w = inp[wn].astype(np.float32)
        inp[bn] = (np.einsum('lf,lfm->lm', b1, w) + inp[bn]).astype(np.float32)
        inp[wn] = (g1 * w).astype(np.float32)
    g2 = inp['ln2_w'][:, :, None]
    w = inp['ff_w1'].astype(np.float32)
    inp['ff_b1'] = (np.einsum('lf,lfm->lm', inp['ln2_b'], w) + inp['ff_b1']).astype(np.float32)
    inp['ff_w1'] = (g2 * w).astype(np.float32)
    w = inp['h_w1'].astype(np.float32)
    inp['h_b1'] = (inp['h_ln_b'] @ w + inp['h_b1']).astype(np.float32)
    inp['h_w1'] = (inp['h_ln_w'][:, None] * w).astype(np.float32)

    skip_res_bias = not (np.any(inp['bo']) or np.any(inp['ff_b2']))
    skip_qkv_bias = not (np.any(inp['bq']) or np.any(inp['bk'])
                         or np.any(inp['bv']))

    sig = inp['sig_n'].astype(np.float32)
    x = np.where(np.isfinite(sig), sig, 0.0)
    x = np.where(x == -1.0, 0.0, x).astype(np.float32)
    valid = np.any(sig != -1.0, axis=1)                # [16, 1000]

    xh = np.zeros((B, 4, L + 8), np.float32)
    xh[:, :, 4:4 + L] = x
    t1full = np.zeros((B, 36, NPAD), np.float32)
    for t in range(9):
        t1full[:, 4 * t:4 * t + 4, 0:L] = xh[:, :, t:t + L]

    meta = inp['meta'].astype(np.int64)
    e_chr = inp['emb_chr'][np.clip(meta[:, 2], 0, 22)]
    e_gene = inp['emb_gene'][np.maximum(inp['gene_id'].astype(np.int64), 0)]
    e_exon = inp['emb_exon'][np.clip(inp['exon_id'].astype(np.int64), 0, 128)]
    e_ctx = (e_chr + e_gene + e_exon).astype(np.float32)

    pos = np.arange(L, dtype=np.float32)[:, None]
    div = np.exp(np.arange(0, D, 2, dtype=np.float32) * (-np.log(10000.0) / D))
    pe = np.zeros((L, D), np.float32)
    pe[:, 0::2] = np.sin(pos * div)
    pe[:, 1::2] = np.cos(pos * div)

    ADD = np.zeros((B, D, NPAD), np.float32)
    ADD[:, :, 0] = inp['cls'][0, 0][None, :] + e_ctx
    ADD[:, :, 1:1 + L] = pe.T[None] + e_ctx[:, :, None]

    w1t = np.zeros((36, 64), np.float32)
    for t in range(9):
        w1t[4 * t:4 * t + 4] = inp['conv1_w'][:, :, t].T
    w2t = np.ascontiguousarray(inp['conv2_w'].transpose(1, 2, 0)).astype(np.float32)
    w3t = np.ascontiguousarray(inp['conv3_w'][:, :, 0].T).astype(np.float32)
    gmt = np.zeros((8, P), np.float32)
    for p in range(P):
        gmt[p // 16, p] = 1.0
    projt = np.ascontiguousarray((inp['proj'] * DN).transpose(0, 2, 1)).astype(np.float32)

    bf = lambda a: np.ascontiguousarray(np.asarray(a, np.float32).astype(ml_dtypes.bfloat16))
    shared = dict(
        w1t=w1t, w2t=w2t, w3t=w3t, gmt=gmt, projt=projt,
        wqb=bf(inp['wq']), wkb=bf(inp['wk']), wvb=bf(inp['wv']),
        wob=bf(inp['wo']),
        f1b=bf(inp['ff_w1']), f2b=bf(inp['ff_w2']),
        hw1b=bf(inp['h_w1']),
        hw2=np.ascontiguousarray(inp['h_w2'], dtype=np.float32),
    )

    in_maps = []
    for c in range(N_CORES):
        b0 = 2 * c
        real01 = np.zeros((P, 16), np.float32)
        vm01 = np.zeros((P, 16), np.float32)
        for s in range(2):
            for n in range(NPAD):
                t, row = n // P, n % P
                if n <= L:
                    real01[row, s * 8 + t] = 1.0
                    if n == 0 or valid[b0 + s, n - 1]:
                        vm01[row, s * 8 + t] = 1.0
        chw = pack_chw(inp, real01, vm01)
        in_maps.append(dict(
            shared,
            xh=np.ascontiguousarray(t1full[b0:b0 + 2]),
            addt=np.ascontiguousarray(ADD[b0:b0 + 2]),
            chw=chw,
        ))

    nc = bacc.Bacc()
    build(nc, skip_res_bias=skip_res_bias, skip_qkv_bias=skip_qkv_bias)
    nc.finalize()
    res = run_bass_kernel_spmd(nc, in_maps, list(range(N_CORES)))
    global LAST_RESULT
    LAST_RESULT = res
    out = np.concatenate([np.asarray(res.results[c]['o']).reshape(2)
                          for c in range(N_CORES)])
    return out.astype(np.float32)


LAST_RESULT = None


if __name__ == '__main__':
    import reference
    inputs = {k: np.asarray(v) for k, v in reference.setup_inputs().items()}
    got = kernel(**inputs)
    print('kernel out:', got)



# revision 58
# speedup vs baseline: 1.3307x; 1.0072x over previous
"""Trainium2 Bass kernel for nn_CNVRegressor (CNN tokenizer + 5-layer Performer + head).

Sharding: data-parallel over batch B=16 across 8 cores (2 samples/core).
Layout: feature-major activations [D on partitions, tokens on free].
Per-sample sequence padded 1001 -> 1024; two samples side by side -> [512, 2048].

Single-pass FAVOR+: kp = exp(ddk - diagk) is computed unstabilized (safe in
f32/bf16 since |ddk| <~ 10); the reference's global key-stab enters ONLY via
the eps-term coefficient gamma = eps*exp(stab_g). The per-core max is taken as
a cheap byproduct of the kp tiles (DVE max + log), AllReduce(max)'d across the
8 cores while the q-side computes, then folded into ctxT.

The query-side eps term of the reference perturbs the final output by only
~2e-4 relative (verified against the oracle in f64) because the per-query
stabilizer makes exp(dd-stab) peak at 1 >> eps; it is dropped here, which
removes the per-query colmax (a gpsimd all-reduce per head-sample) and the
rank-1 eps correction entirely.

LayerNorm gamma/beta of ln1/ln2/h_ln are folded into the consumer weight
matrices host-side (W' = gamma*W, b' = beta@W + b), so the device LN only
computes (x-mu)*rsigma.

Self-contained: hardcodes all shapes; host does only input prep / sharding
(cleanup, halo pad, embedding row gather, PE table, bias/mask packing).
"""
import math
from contextlib import ExitStack

import ml_dtypes
import numpy as np

import concourse.bass as bass
import concourse.bacc as bacc
import concourse.tile as tile
from concourse import mybir
from concourse.bass_isa import ReduceOp
from concourse.bass_utils import run_bass_kernel_spmd
from concourse.masks import make_identity

F32 = mybir.dt.float32
F32R = mybir.dt.float32r
BF16 = mybir.dt.bfloat16
AF = mybir.ActivationFunctionType
OP = mybir.AluOpType
AX = mybir.AxisListType

P = 128
D = 512
DH = 64
DEPTH = 5
L = 1000
M = 266
NPAD = 1024
N2 = 2 * NPAD
NT = NPAD // P            # 8 token tiles per sample
DC = D // P               # 4 feature chunks
BLK = 512
DN = DH ** -0.25
DNS = DN * math.sqrt(0.5)
LN_EPS = 1e-5
GN_EPS = 1e-5
LOG_EPS = float(np.log(1e-4))
MCH = ((0, 128), (128, 128), (256, 10))   # m-chunks of 266
N_CORES = 8


def build_cols():
    cols, idx = {}, 0

    def a(name, n):
        nonlocal idx
        cols[name] = idx
        idx += n

    for n in ('gn_w', 'gn_b', 'lnt_w', 'lnt_b'):
        a(n, 4)
    a('b1', 1); a('b2', 1); a('b3', 4); a('gmask', 8)
    a('real', 16); a('stabb', 16); a('vmask', 16)
    a('real2', 32); a('vmask2', 32)   # (s, t, hh) layout: col = s*16 + t*2 + hh
    for n in ('hln_w', 'hln_b', 'hb1'):
        a(n, 4)
    a('hb2', 1)
    a('cln', 1)
    a('clog', 1)
    a('ctiny', 1)
    for l in range(DEPTH):
        for n in ('ln1w', 'ln1b', 'ln2w', 'ln2b', 'bq', 'bk', 'bv', 'bo', 'fb2'):
            a(f'{n}{l}', 4)
        a(f'fb1{l}', 16)
    return cols, idx


COLS, NCOL = build_cols()


def pack_chw(inp, real01, vmask01):
    chw = np.zeros((P, NCOL), np.float32)

    def put(name, vec):
        vec = np.asarray(vec, np.float32).reshape(-1)
        c0 = COLS[name]
        for c in range((len(vec) + P - 1) // P):
            seg = vec[c * P:(c + 1) * P]
            chw[:len(seg), c0 + c] = seg

    put('gn_w', inp['gn_w']); put('gn_b', inp['gn_b'])
    put('lnt_w', inp['lnt_w']); put('lnt_b', inp['lnt_b'])
    put('b1', inp['conv1_b']); put('b2', inp['conv2_b']); put('b3', inp['conv3_b'])
    gm = np.zeros((P, 8), np.float32)
    for p in range(P):
        gm[p, p // 16] = 1.0
    chw[:, COLS['gmask']:COLS['gmask'] + 8] = gm
    chw[:, COLS['real']:COLS['real'] + 16] = real01
    chw[:, COLS['stabb']:COLS['stabb'] + 16] = (real01 - 1.0) * 1e30
    chw[:, COLS['vmask']:COLS['vmask'] + 16] = vmask01
    chw[:, COLS['real2']:COLS['real2'] + 32] = np.repeat(real01, 2, axis=1)
    chw[:, COLS['vmask2']:COLS['vmask2'] + 32] = np.repeat(vmask01, 2, axis=1)
    put('hln_w', inp['h_ln_w']); put('hln_b', inp['h_ln_b'])
    put('hb1', inp['h_b1']); put('hb2', inp['h_b2'])
    chw[:, COLS['cln']] = LN_EPS
    chw[:, COLS['clog']] = LOG_EPS
    chw[:, COLS['ctiny']] = 1e-30
    for l in range(DEPTH):
        put(f'ln1w{l}', inp['ln1_w'][l]); put(f'ln1b{l}', inp['ln1_b'][l])
        put(f'ln2w{l}', inp['ln2_w'][l]); put(f'ln2b{l}', inp['ln2_b'][l])
        put(f'bq{l}', inp['bq'][l]); put(f'bk{l}', inp['bk'][l])
        put(f'bv{l}', inp['bv'][l]); put(f'bo{l}', inp['bo'][l])
        put(f'fb1{l}', inp['ff_b1'][l]); put(f'fb2{l}', inp['ff_b2'][l])
    return chw


def blocks(width, bs=BLK, off0=0):
    out, off = [], 0
    while off < width:
        out.append((off0 + off, min(bs, width - off)))
        off += bs
    return out


# ---------------------------------------------------------------- device build
def build(nc, skip_res_bias=False, skip_qkv_bias=False):
    dp = lambda n, sh, dt=F32: nc.declare_dram_parameter(n, sh, dt, isOutput=False)
    xh_d = dp('xh', (2, 36, NPAD))
    add_d = dp('addt', (2, D, NPAD))
    w1t_d = dp('w1t', (36, 64))
    w2t_d = dp('w2t', (64, 9, 128))
    w3t_d = dp('w3t', (128, D))
    chw_d = dp('chw', (P, NCOL))
    gmt_d = dp('gmt', (8, P))
    proj_d = dp('projt', (DEPTH, DH, M))
    wq_d = dp('wqb', (DEPTH, D, D), BF16)
    wk_d = dp('wkb', (DEPTH, D, D), BF16)
    wv_d = dp('wvb', (DEPTH, D, D), BF16)
    wob_d = dp('wob', (DEPTH, D, D), BF16)
    f1_d = dp('f1b', (DEPTH, D, 4 * D), BF16)
    f2_d = dp('f2b', (DEPTH, 4 * D, D), BF16)
    hw1_d = dp('hw1b', (D, D), BF16)
    hw2_d = dp('hw2', (D, 1))
    out_d = nc.declare_dram_parameter('o', (1, 2), F32, isOutput=True)

    with tile.TileContext(nc) as tc, ExitStack() as ctx:
        const = ctx.enter_context(tc.tile_pool(name='const', bufs=1))
        sp = ctx.enter_context(tc.tile_pool(name='sp', bufs=1))
        actp = ctx.enter_context(tc.tile_pool(name='actp', bufs=1))
        wpp = ctx.enter_context(tc.tile_pool(name='wpp', bufs=2))
        fp = ctx.enter_context(tc.tile_pool(name='fp', bufs=2))
        kvp = ctx.enter_context(tc.tile_pool(name='kvp', bufs=3))
        dram = ctx.enter_context(tc.tile_pool(name='dram', bufs=2, space='DRAM'))
        pmm = ctx.enter_context(tc.tile_pool(name='pmm', bufs=4, space='PSUM'))
        ps1 = ctx.enter_context(tc.tile_pool(name='ps1', bufs=2, space='PSUM'))

        # ---- constants
        chw = const.tile([P, NCOL], F32, name='chw')
        nc.sync.dma_start(chw[:], chw_d[:])
        cw = lambda name, off=0: chw[:, COLS[name] + off:COLS[name] + off + 1]
        cwp = lambda name, parts: chw[0:parts, COLS[name]:COLS[name] + 1]
        ident = const.tile([P, P], F32, name='ident')
        make_identity(nc, ident)
        identb = const.tile([P, P], BF16, name='identb')
        nc.vector.tensor_copy(identb[:], ident[:])
        ones = const.tile([P, 1], F32, name='ones')
        nc.vector.memset(ones[:], 1.0)
        onesb = const.tile([P, 1], BF16, name='onesb')
        nc.vector.memset(onesb[:], 1.0)
        ones2b = const.tile([P, 2], BF16, name='ones2b')
        nc.vector.memset(ones2b[:], 0.0)
        nc.vector.memset(ones2b[0:DH, 0:1], 1.0)
        nc.vector.memset(ones2b[DH:P, 1:2], 1.0)
        onesD = const.tile([P, 1], F32, name='onesD')
        nc.vector.memset(onesD[:], 1.0 / D)
        onesDb = const.tile([P, 1], BF16, name='onesDb')
        nc.vector.memset(onesDb[:], 1.0 / D)
        gmt = const.tile([8, P], F32, name='gmt')
        nc.sync.dma_start(gmt[:], gmt_d[:])
        w1t = const.tile([36, 64], F32, name='w1t')
        nc.sync.dma_start(w1t[:], w1t_d[:])
        w2t = const.tile([64, 9, 128], F32, name='w2t')
        nc.sync.dma_start(w2t[:], w2t_d[:])
        w3t = const.tile([128, D], F32, name='w3t')
        nc.sync.dma_start(w3t[:], w3t_d[:])

        S = [sp.tile([P, N2], F32, tag=f'S{c}', name=f'S{c}') for c in range(DC)]

        # -------------------------------------------------------- shared LN
        def ln_fm(X, wc, bc, col0, width, ytag):
            """Per-token LN over the 512 partition dim (feature-major).

            X tiles are f32; returns bf16 normed tiles."""
            Y = [actp.tile([P, N2], BF16, tag=f'{ytag}{c}', name=f'{ytag}{c}')
                 for c in range(DC)]
            for c in range(DC):
                nc.vector.tensor_mul(Y[c][:, col0:col0 + width],
                                     X[c][:, col0:col0 + width],
                                     X[c][:, col0:col0 + width])
            srow = fp.tile([1, N2], F32, tag='srow', bufs=1, name='srow')
            qrow = fp.tile([1, N2], F32, tag='qrow', bufs=1, name='qrow')
            mrow, vrow = srow, qrow
            for o, bw in blocks(width, BLK, col0):
                ps = ps1.tile([1, BLK], F32, tag='ps1', name='ps')
                pq = ps1.tile([1, BLK], F32, tag='ps1', name='pq')
                for c in range(DC):
                    xbt = kvp.tile([P, BLK], BF16, tag='xbt', bufs=2, name='xbt')
                    nc.any.tensor_copy(xbt[:, :bw], X[c][:, o:o + bw])
                    nc.tensor.matmul(ps[:, :bw], onesDb[:], xbt[:, :bw],
                                     start=(c == 0), stop=(c == DC - 1))
                    nc.tensor.matmul(pq[:, :bw], onesDb[:], Y[c][:, o:o + bw],
                                     start=(c == 0), stop=(c == DC - 1))
                nc.any.tensor_copy(mrow[:, o:o + bw], ps[:, :bw])
                nc.any.tensor_copy(vrow[:, o:o + bw], pq[:, :bw])
            MU = actp.tile([P, N2], F32, tag='MU', name='MU')
            RS = actp.tile([P, N2], F32, tag='RS', name='RS')
            trow = MU[0:1, :]
            # rows + broadcast + apply fully pipelined per 512-block: block
            # 0's normalized output (and thus the first consumer matmul) is
            # ready while blocks 1-3 stats are still accumulating
            for o, bw in blocks(width, BLK, col0):
                nc.vector.tensor_mul(trow[:, o:o + bw], mrow[:, o:o + bw],
                                     mrow[:, o:o + bw])
                nc.vector.tensor_sub(vrow[:, o:o + bw], vrow[:, o:o + bw],
                                     trow[:, o:o + bw])
                nc.scalar.activation(vrow[:, o:o + bw], vrow[:, o:o + bw],
                                     AF.Ln, bias=cwp('cln', 1))
                nc.scalar.activation(vrow[:, o:o + bw], vrow[:, o:o + bw],
                                     AF.Exp, scale=-0.5)
                nc.gpsimd.partition_broadcast(MU[:, o:o + bw], mrow[:, o:o + bw], P)
                nc.gpsimd.partition_broadcast(RS[:, o:o + bw], vrow[:, o:o + bw], P)
                for c in range(DC):
                    nc.vector.tensor_sub(Y[c][:, o:o + bw], X[c][:, o:o + bw],
                                         MU[:, o:o + bw])
                    nc.vector.tensor_mul(Y[c][:, o:o + bw], Y[c][:, o:o + bw],
                                         RS[:, o:o + bw])
                    if wc is not None:
                        nc.scalar.activation(Y[c][:, o:o + bw], Y[c][:, o:o + bw],
                                             AF.Identity,
                                             scale=cw(wc, c), bias=cw(bc, c))
            return Y

        # full-width projection: dst[128, N2] = (w^T y) + bias, both heads of
        # a pair. kc-outer so each stationary is loaded once per 4 blocks.
        def proj_mm(wd, l, hp, bn, dst):
            wt = wpp.tile([P, DC, P], BF16, tag='wpq', name='wt')
            nc.sync.dma_start(
                wt[:], wd[l, :, hp * P:(hp + 1) * P]
                .rearrange('(kc p) m -> p kc m', p=P))
            bias = chw[:, COLS[f'{bn}{l}'] + hp:COLS[f'{bn}{l}'] + hp + 1]
            pms = [pmm.tile([P, BLK], F32, tag='pmm', name=f'pm{bi}')
                   for bi in range(4)]
            for kc in range(DC):
                for bi, (b, bw) in enumerate(blocks(N2)):
                    nc.tensor.matmul(pms[bi][:], wt[:, kc, :], y[kc][:, b:b + bw],
                                     start=(kc == 0), stop=(kc == DC - 1))
            for bi, (b, bw) in enumerate(blocks(N2)):
                if skip_qkv_bias:
                    # bias known zero: plain copy, schedulable on ACT or DVE
                    nc.any.tensor_copy(dst[:, b:b + bw], pms[bi][:])
                else:
                    nc.scalar.activation(dst[:, b:b + bw], pms[bi][:],
                                         AF.Identity, bias=bias)

        # -------------------------------------------------------- tokenizer
        for s in range(2):
            for c in range(DC):
                nc.sync.dma_start(S[c][:, s * NPAD:(s + 1) * NPAD],
                                  add_d[s, c * P:(c + 1) * P, :])
        for s in range(2):
            t1 = fp.tile([36, NPAD], F32, tag='tokA' if s == 0 else 'vh20',
                         bufs=1, name='t1')
            nc.sync.dma_start(t1[:], xh_d[s])
            y1h = fp.tile([64, L + 8], F32, tag='tokB' if s == 0 else 'vh21',
                          bufs=1, name='y1h')
            nc.vector.memset(y1h[:], 0.0)
            for o, bw in blocks(L):
                p1 = pmm.tile([64, BLK], F32, tag='pmm', name='p1')
                nc.tensor.matmul(p1[:, :bw], w1t[:], t1[:, o:o + bw],
                                 start=True, stop=True)
                nc.scalar.activation(y1h[:, 4 + o:4 + o + bw], p1[:, :bw], AF.Gelu,
                                     bias=chw[0:64, COLS['b1']:COLS['b1'] + 1])
            y2 = fp.tile([P, NPAD], F32, tag='tokA' if s == 0 else 'vh22',
                         bufs=1, name='y2')
            for o, bw in blocks(L):
                p2 = pmm.tile([P, BLK], F32, tag='pmm', name='p2')
                for t in range(9):
                    nc.tensor.matmul(p2[:, :bw], w2t[:, t, :],
                                     y1h[:, t + o:t + o + bw],
                                     start=(t == 0), stop=(t == 8))
                nc.scalar.activation(y2[:, o:o + bw], p2[:, :bw], AF.Gelu,
                                     bias=cw('b2'))
            # sample 1 borrows the (still dead) kh2 layer slots so the two
            # samples' tokenizer passes don't serialize on shared tiles
            x3 = [actp.tile([P, NPAD], F32, tag=f'A{c}', name=f'x3{c}')
                  if s == 0 else
                  fp.tile([P, NPAD], F32, tag=f'kh2{c}', bufs=1, name=f'x3{c}')
                  for c in range(DC)]
            for c in range(DC):
                for o, bw in blocks(L):
                    p3 = pmm.tile([P, BLK], F32, tag='pmm', name='p3')
                    nc.tensor.matmul(p3[:, :bw], w3t[:, c * P:(c + 1) * P],
                                     y2[:, o:o + bw], start=True, stop=True)
                    nc.scalar.activation(x3[c][:, o:o + bw], p3[:, :bw],
                                         AF.Identity, bias=cw('b3', c))
            # GroupNorm(32, 512) over [16ch x 1000]
            stats = fp.tile([P, 8], F32, tag='gstats', name='stats')
            sqt = fp.tile([P, NPAD], F32, tag='tokB' if s == 0 else 'vh23',
                          bufs=1, name='sqt')
            for c in range(DC):
                nc.vector.tensor_reduce(stats[:, c:c + 1], x3[c][:, 0:L], AX.X, OP.add)
                nc.vector.tensor_mul(sqt[:, 0:L], x3[c][:, 0:L], x3[c][:, 0:L])
                nc.vector.tensor_reduce(stats[:, 4 + c:5 + c], sqt[:, 0:L], AX.X, OP.add)
            pg = ps1.tile([8, 8], F32, tag='ps1', name='pg')
            nc.tensor.matmul(pg[:], chw[:, COLS['gmask']:COLS['gmask'] + 8],
                             stats[:], start=True, stop=True)
            gs = fp.tile([8, 8], F32, tag='gs', name='gs')
            nc.vector.tensor_scalar_mul(gs[:], pg[:], 1.0 / (16 * L))
            gm2 = fp.tile([8, 4], F32, tag='gm2', name='gm2')
            nc.vector.tensor_mul(gm2[:], gs[:, 0:4], gs[:, 0:4])
            nc.vector.tensor_sub(gs[:, 4:8], gs[:, 4:8], gm2[:])
            nc.scalar.activation(gs[:, 4:8], gs[:, 4:8], AF.Ln, bias=cwp('cln', 8))
            nc.scalar.activation(gs[:, 4:8], gs[:, 4:8], AF.Exp, scale=-0.5)
            pb = ps1.tile([P, 8], F32, tag='ps1', name='pb')
            nc.tensor.matmul(pb[:], gmt[:], gs[:], start=True, stop=True)
            cstat = fp.tile([P, 8], F32, tag='cstat', name='cstat')
            nc.vector.tensor_copy(cstat[:], pb[:])
            for c in range(DC):
                nc.vector.tensor_scalar(x3[c][:, 0:L], x3[c][:, 0:L],
                                        cstat[:, c:c + 1], cstat[:, 4 + c:5 + c],
                                        OP.subtract, OP.mult)
                nc.scalar.activation(x3[c][:, 0:L], x3[c][:, 0:L], AF.Identity,
                                     scale=cw('gn_w', c), bias=cw('gn_b', c))
            tok = ln_fm(x3, 'lnt_w', 'lnt_b', 0, L, 'y')
            b0 = s * NPAD
            for c in range(DC):
                nc.vector.tensor_add(S[c][:, b0 + 1:b0 + 1 + L],
                                     S[c][:, b0 + 1:b0 + 1 + L], tok[c][:, 0:L])

        # -------------------------------------------------------- layers
        for l in range(DEPTH):
            projT = fp.tile([DH, M], F32, tag='projT', name='projT')
            nc.sync.dma_start(projT[:], proj_d[l])
            # duplicated into both partition halves so head-1 slices
            # (base partition 64) can pair with it in matmuls; the second
            # half is filled by DMA (engines can't shift partitions)
            projTb = fp.tile([P, M], BF16, tag='projTb', name='projTb')
            nc.vector.tensor_copy(projTb[0:DH, :], projT[:])
            nc.sync.dma_start(projTb[DH:P, :], projTb[0:DH, :])

            y = ln_fm(S, None, None, 0, N2, 'y')  # gamma/beta folded into wq/wk/wv

            # per-layer k-side context accumulators [65, 272] x 16 head-samples
            ctxE = fp.tile([65, 16, 272], BF16, tag='ctxE', bufs=1, name='ctxE')
            smax = fp.tile([P, 16], F32, tag='smax', name='smax')

            # ---- phase K: all K/V projections first (dense GEMM block),
            # then all diag/e^{+-diag} precomputation, then the light
            # per-head-sample kp/ctx loops with every input already ready —
            # this keeps the PE warm and avoids ACT-FIFO head-of-line
            # blocking between the e+- chains and the kp exps.
            KH, VH = [], []
            for hp in range(4):
                kh2 = fp.tile([P, N2], BF16, tag=f'kh2{hp}', bufs=1,
                              name=f'kh2{hp}')
                vh2 = fp.tile([P, N2], BF16, tag=f'vh2{hp}', bufs=1,
                              name=f'vh2{hp}')
                proj_mm(wk_d, l, hp, 'bk', kh2)
                proj_mm(wv_d, l, hp, 'bv', vh2)
                KH.append(kh2)
                VH.append(vh2)
            # diag_k for both heads at once: pd8[:, t, hh] = sum_d (DNS*k)^2
            # via N=2 matmuls against the half-ones columns; e^{+-diag} is
            # folded into the v1 scale / vsum indicator instead of biasing
            # the kp exp (kp = e^{dd} directly, pads -> 0 via the constant
            # stabb bias).
            EE = {}
            for hp in range(4):
                # shares the tokenizer's (long dead) tokA slot to save SBUF
                sq2 = fp.tile([P, N2], BF16, tag='tokA', bufs=1, name='sq2')
                nc.scalar.activation(sq2[:], KH[hp][:], AF.Square, scale=DNS)
                for s in range(2):
                    base = s * NPAD
                    pd8 = ps1.tile([P, 8, 2], F32, tag='ps1', name='pd8')
                    for t in range(NT):
                        csl = slice(base + t * P, base + (t + 1) * P)
                        nc.tensor.matmul(pd8[:, t, :], sq2[:, csl], ones2b[:],
                                         start=True, stop=True)
                    d8 = fp.tile([P, 16], F32, tag='d8', bufs=2, name='d8')
                    nc.vector.tensor_copy(d8[:], pd8.rearrange('p t h -> p (t h)'))
                    epr = fp.tile([P, 16], F32, tag='epr', bufs=1, name='epr')
                    enr = fp.tile([P, 16], F32, tag='enr', bufs=1, name='enr')
                    nc.scalar.activation(epr[:], d8[:], AF.Exp)
                    nc.scalar.activation(enr[:], d8[:], AF.Exp, scale=-1.0)
                    epos = fp.tile([P, 16], F32, tag=f'epos{hp}{s}', bufs=1,
                                   name='epos')
                    eneg = fp.tile([P, 16], F32, tag=f'eneg{hp}{s}', bufs=1,
                                   name='eneg')
                    erel = fp.tile([P, 16], F32, tag=f'erel{hp}{s}', bufs=1,
                                   name='erel')
                    c2 = COLS['real2'] + s * 16
                    cv = COLS['vmask2'] + s * 16
                    nc.vector.tensor_mul(epos[:], epr[:], chw[:, c2:c2 + 16])
                    nc.vector.tensor_mul(eneg[:], enr[:], chw[:, cv:cv + 16])
                    nc.vector.tensor_mul(erel[:], enr[:], chw[:, c2:c2 + 16])
                    EE[hp, s] = (epos, eneg, erel)
            for hp in range(4):
                kh2, vh2 = KH[hp], VH[hp]
                for hh in range(2):
                    hsl = slice(hh * DH, (hh + 1) * DH)
                    for s in range(2):
                        base = s * NPAD
                        idx = hp * 4 + hh * 2 + s
                        epos, eneg, erel = EE[hp, s]
                        # --- kp tiles + ctx'^T [65, 267] accumulation
                        pctx = ps1.tile([65, 272], F32, tag='psx', bufs=2,
                                        name='pctx')
                        rm8 = fp.tile([P, 8], F32, tag='rm8', name='rm8')
                        for t in range(NT):
                            csl = slice(base + t * P, base + (t + 1) * P)
                            tcol = t * 2 + hh
                            pdk = ps1.tile([P, 272], F32, tag='ps1', name='pdk')
                            nc.tensor.matmul(pdk[:, 0:M], kh2[hsl, csl],
                                             projTb[hsl, :], start=True, stop=True)
                            kp = kvp.tile([P, 272], BF16, tag='kp', bufs=4,
                                          name='kp')
                            nc.scalar.activation(kp[:, 0:M], pdk[:, 0:M], AF.Exp,
                                                 bias=cw('stabb', s * 8 + t))
                            nc.vector.tensor_copy(kp[:, M:M + 1],
                                                  epos[:, tcol:tcol + 1])
                            nc.vector.tensor_reduce(rm8[:, t:t + 1], kp[:, 0:M],
                                                    AX.X, OP.max)
                            pvt = pmm.tile([P, 64], BF16, tag='pmm', name='pvt')
                            nc.tensor.transpose(pvt[:], vh2[hsl, csl],
                                                identb[hsl, hsl])
                            v1 = kvp.tile([P, 65], BF16, tag='v1', bufs=4,
                                          name='v1')
                            nc.vector.tensor_scalar_mul(v1[:, 0:64], pvt[:],
                                                        eneg[:, tcol:tcol + 1])
                            nc.vector.tensor_copy(v1[:, 64:65],
                                                  erel[:, tcol:tcol + 1])
                            nc.tensor.matmul(pctx[:, 0:M + 1], v1[:], kp[:, 0:M + 1],
                                             start=(t == 0), stop=(t == NT - 1))
                        nc.vector.tensor_copy(ctxE[:, idx, 0:M + 1],
                                              pctx[:, 0:M + 1])
                        # --- local stab byproduct: kp is e^{dd} directly, so
                        # smax is just the running max (pad rows stay 0).
                        nc.vector.tensor_reduce(smax[:, idx:idx + 1], rm8[:],
                                                AX.X, OP.max)

            # ---- global key-stab: AllReduce(max) across the 8 cores.
            # Overlaps with the q-side below (consumed only at ctxT/vsc).
            # high_priority jumps this latency chain ahead of the queued
            # per-head-sample gpsimd/DVE work in the engine FIFOs.
            with tc.high_priority():
                sfin = fp.tile([P, 1], F32, tag='sfin', name='sfin')
                nc.vector.tensor_reduce(sfin[:], smax[:], AX.X, OP.max)
                nc.gpsimd.partition_all_reduce(sfin[:], sfin[:], P, ReduceOp.max)
                bin_ = dram.tile([P, 1], F32, name='bin')
                bout = dram.tile([P, 1], F32, name='bout')
                nc.sync.dma_start(bin_[:], sfin[:])
                nc.gpsimd.collective_compute(
                    'AllReduce', OP.max,
                    replica_groups=[list(range(N_CORES))],
                    ins=[bin_.opt()], outs=[bout.opt()])
                stabg = fp.tile([P, 1], F32, tag='stabg', name='stabg')
                nc.sync.dma_start(stabg[:], bout[:])
                # stabg already holds e^{stab_g}; gamma = eps * e^{stab_g}
                ceps65 = fp.tile([65, 1], F32, tag='ceps65', name='ceps65')
                nc.vector.tensor_scalar_mul(ceps65[:], stabg[0:65, :], 1e-4)

            # ---- phase Q: q projection, qp = exp(ddq), num/den -> A.
            # The reference's query-side eps term is dropped (rel effect
            # ~2e-4, verified vs the f64 oracle); the key-side eps enters
            # via ctxT below.
            A = [actp.tile([P, N2], BF16, tag=f'A{c}', name=f'Aa{c}')
                 for c in range(DC)]
            for hp in range(4):
                qh2 = fp.tile([P, N2], BF16, tag='qh2', bufs=2, name='qh2')
                proj_mm(wq_d, l, hp, 'bq', qh2)
                for hh in range(2):
                    hsl = slice(hh * DH, (hh + 1) * DH)
                    for s in range(2):
                        base = s * NPAD
                        idx = hp * 4 + hh * 2 + s
                        # --- qp = exp(ddq), feature-major
                        qp = [fp.tile([P, NPAD], BF16, tag='qp0', bufs=2, name='qp0'),
                              fp.tile([P, NPAD], BF16, tag='qp1', bufs=2, name='qp1'),
                              fp.tile([10, NPAD], BF16, tag='qp2', bufs=2, name='qp2')]
                        for ci, (m0, mw) in enumerate(MCH):
                            for b, bw in blocks(NPAD):
                                pdq = pmm.tile([P, BLK], F32, tag='pmm', name='pdq')
                                nc.tensor.matmul(
                                    pdq[0:mw, :], projTb[hsl, m0:m0 + mw],
                                    qh2[hsl, base + b:base + b + bw],
                                    start=True, stop=True)
                                nc.scalar.activation(qp[ci][0:mw, b:b + bw],
                                                     pdq[0:mw, :], AF.Exp)
                        # --- ctxT = ctxE + gamma * vsum; -> [266, 65] chunks
                        vsc = fp.tile([65, 1], F32, tag='vsc', name='vsc')
                        ctxT = fp.tile([65, M], BF16, tag='ctxT', name='ctxT')
                        if idx < 4:
                            # first head-samples jump the DVE queue so the
                            # post-collective chain restarts the PE sooner
                            with tc.high_priority():
                                nc.vector.tensor_mul(vsc[:], ctxE[:, idx, M:M + 1],
                                                     ceps65[:])
                                nc.vector.tensor_scalar(ctxT[:], ctxE[:, idx, 0:M],
                                                        vsc[:], None, OP.add)
                        else:
                            nc.vector.tensor_mul(vsc[:], ctxE[:, idx, M:M + 1],
                                                 ceps65[:])
                            nc.vector.tensor_scalar(ctxT[:], ctxE[:, idx, 0:M],
                                                    vsc[:], None, OP.add)
                        ctx_sb = []
                        for ci, (m0, mw) in enumerate(MCH):
                            ptc = ps1.tile([P, 65], BF16, tag='ps1', name='ptc')
                            nc.tensor.transpose(ptc[0:mw, :], ctxT[:, m0:m0 + mw],
                                                identb[0:65, 0:65])
                            csb = fp.tile([P, 65], BF16, tag=f'ctx{ci}', name=f'c{ci}')
                            nc.any.tensor_copy(csb[0:mw, :], ptc[0:mw, :])
                            ctx_sb.append(csb)
                        # --- num_den [65, n]; rows 0..63 num, row 64 den
                        for b, bw in blocks(NPAD):
                            pnd = ps1.tile([65, BLK], F32, tag='ps1', name='pnd')
                            for ci, (m0, mw) in enumerate(MCH):
                                nc.tensor.matmul(pnd[:], ctx_sb[ci][0:mw, :],
                                                 qp[ci][0:mw, b:b + bw],
                                                 start=(ci == 0), stop=(ci == 2))
                            den = fp.tile([1, BLK], F32, tag='dvb', bufs=2,
                                          name='den')
                            nc.any.tensor_copy(den[:], pnd[64:65, :])
                            dinv = fp.tile([1, BLK], F32, tag='dinv', bufs=2,
                                           name='dinv')
                            nc.vector.reciprocal_approx_fast(dinv[:], den[:])
                            dvb = fp.tile([64, BLK], F32, tag='dvb', bufs=2,
                                          name='dvb')
                            nc.gpsimd.partition_broadcast(dvb[:], dinv[:], 64)
                            nc.vector.tensor_mul(
                                A[hp][hsl, base + b:base + b + bw],
                                pnd[0:64, :], dvb[:])

            # ---- wo: S += A @ wo + bo (kc-outer for stationary reuse)
            for mc in range(DC):
                wt = wpp.tile([P, DC, P], BF16, tag='wpo', name='wto')
                nc.sync.dma_start(
                    wt[:], wob_d[l, :, mc * P:(mc + 1) * P]
                    .rearrange('(kc p) m -> p kc m', p=P))
                pms = [pmm.tile([P, BLK], F32, tag='pmm', name=f'pmo{bi}')
                       for bi in range(4)]
                for kc in range(DC):
                    for bi, (b, bw) in enumerate(blocks(N2)):
                        nc.tensor.matmul(pms[bi][:], wt[:, kc, :],
                                         A[kc][:, b:b + bw],
                                         start=(kc == 0), stop=(kc == DC - 1))
                for bi, (b, bw) in enumerate(blocks(N2)):
                    nc.vector.tensor_add(S[mc][:, b:b + bw], S[mc][:, b:b + bw],
                                         pms[bi][:])
                if not skip_res_bias:
                    nc.scalar.activation(S[mc][:], S[mc][:], AF.Identity,
                                         bias=cw(f'bo{l}', mc))

            # ---- FF in quarters of the 2048 hidden dim (kc-outer)
            y2t = ln_fm(S, None, None, 0, N2, 'y')  # gamma/beta folded into ff_w1
            for q in range(4):
                w1q = fp.tile([P, DC, BLK], BF16, tag='w1q', bufs=1, name='w1q')
                nc.sync.dma_start(
                    w1q[:], f1_d[l, :, q * BLK:(q + 1) * BLK]
                    .rearrange('(kc p) m -> p kc m', p=P))
                w2q = fp.tile([P, DC, BLK], BF16, tag='w2q', bufs=1, name='w2q')
                nc.sync.dma_start(
                    w2q[:], f2_d[l, q * BLK:(q + 1) * BLK, :]
                    .rearrange('(kc p) m -> p kc m', p=P))
                # reuse the dead A tiles (same shape) for the FF hidden
                H = [actp.tile([P, N2], BF16, tag=f'A{mc}', name=f'H{mc}')
                     for mc in range(DC)]
                for mc in range(DC):
                    pms = [pmm.tile([P, BLK], F32, tag='pmm', name=f'pmf1{bi}')
                           for bi in range(4)]
                    for kc in range(DC):
                        for bi, (b, bw) in enumerate(blocks(N2)):
                            nc.tensor.matmul(pms[bi][:],
                                             w1q[:, kc, mc * P:(mc + 1) * P],
                                             y2t[kc][:, b:b + bw],
                                             start=(kc == 0), stop=(kc == DC - 1))
                    for bi, (b, bw) in enumerate(blocks(N2)):
                        nc.scalar.activation(H[mc][:, b:b + bw], pms[bi][:],
                                             AF.Gelu, bias=cw(f'fb1{l}', q * 4 + mc))
                for mc in range(DC):
                    pms = [pmm.tile([P, BLK], F32, tag='pmm', name=f'pmf2{bi}')
                           for bi in range(4)]
                    for kc in range(DC):
                        for bi, (b, bw) in enumerate(blocks(N2)):
                            nc.tensor.matmul(pms[bi][:],
                                             w2q[:, kc, mc * P:(mc + 1) * P],
                                             H[kc][:, b:b + bw],
                                             start=(kc == 0), stop=(kc == DC - 1))
                    for bi, (b, bw) in enumerate(blocks(N2)):
                        nc.vector.tensor_add(S[mc][:, b:b + bw],
                                             S[mc][:, b:b + bw], pms[bi][:])
            for mc in range(DC):
                if not skip_res_bias:
                    nc.scalar.activation(S[mc][:], S[mc][:], AF.Identity,
                                         bias=cw(f'fb2{l}', mc))
                for s in range(2):
                    nc.vector.memset(S[mc][:, s * NPAD + 1 + L:(s + 1) * NPAD], 0.0)

        # -------------------------------------------------------- head
        clsx = [fp.tile([P, 2], F32, tag=f'cls{c}', name=f'cls{c}')
                for c in range(DC)]
        for c in range(DC):
            nc.vector.tensor_copy(clsx[c][:, 0:1], S[c][:, 0:1])
            nc.vector.tensor_copy(clsx[c][:, 1:2], S[c][:, NPAD:NPAD + 1])
        hx = ln_fm(clsx, None, None, 0, 2, 'y')  # gamma/beta folded into h_w1
        hh_t = []
        for mc in range(DC):
            wt = wpp.tile([P, DC, P], BF16, tag='wpq', name='wth')
            nc.sync.dma_start(wt[:], hw1_d[:, mc * P:(mc + 1) * P]
                              .rearrange('(kc p) m -> p kc m', p=P))
            pm = ps1.tile([P, 2], F32, tag='ps1', name='pmh')
            for kc in range(DC):
                nc.tensor.matmul(pm[:], wt[:, kc, :], hx[kc][:, 0:2],
                                 start=(kc == 0), stop=(kc == DC - 1))
            ht = fp.tile([P, 2], F32, tag=f'hh{mc}', name=f'hhd{mc}')
            nc.scalar.activation(ht[:], pm[:], AF.Gelu, bias=cw('hb1', mc))
            hh_t.append(ht)
        wt2 = fp.tile([P, DC, 1], F32, tag='wt2', name='wt2')
        nc.sync.dma_start(wt2[:], hw2_d[:, :].rearrange('(kc p) m -> p kc m', p=P))
        po = ps1.tile([1, 2], F32, tag='ps1', name='po')
        for kc in range(DC):
            nc.tensor.matmul(po[:], wt2[:, kc, :], hh_t[kc][:, 0:2],
                             start=(kc == 0), stop=(kc == DC - 1))
        osb = fp.tile([1, 2], F32, tag='osb', name='osb')
        nc.scalar.activation(osb[:], po[:], AF.Identity,
                             bias=chw[0:1, COLS['hb2']:COLS['hb2'] + 1])
        nc.sync.dma_start(out_d[:], osb[:])

    return nc


# ---------------------------------------------------------------- host wrapper
def kernel(**inputs):
    inp = {k: np.asarray(v) for k, v in inputs.items()}
    B = inp['sig_n'].shape[0]
    assert B == 16, f'expected B=16, got {B}'

    # Fold ln1/ln2/h_ln gamma/beta into the consumer weights:
    #   W' = gamma[:,None]*W ; b' = beta @ W + b
    g1 = inp['ln1_w'][:, :, None]
    b1 = inp['ln1_b']
    for wn, bn in (('wq', 'bq'), ('wk', 'bk'), ('wv', 'bv')):
        w = inp[wn].astype(np.float32)
        inp[bn] = (np.einsum('lf,lfm->lm', b1, w) + inp[bn]).astype(np.float32)
        inp[wn] = (g1 * w).astype(np.float32)
    g2 = inp['ln2_w'][:, :, None]
    w = inp['ff_w1'].astype(np.float32)
    inp['ff_b1'] = (np.einsum('lf,lfm->lm', inp['ln2_b'], w) + inp['ff_b1']).astype(np.float32)
    inp['ff_w1'] = (g2 * w).astype(np.float32)
    w = inp['h_w1'].astype(np.float32)
    inp['h_b1'] = (inp['h_ln_b'] @ w + inp['h_b1']).astype(np.float32)
    inp['h_w1'] = (inp['h_ln_w'][:, None] * w).astype(np.float32)

    skip_res_bias = not (np.any(inp['bo']) or np.any(inp['ff_b2']))
    skip_qkv_bias = not (np.any(inp['bq']) or np.any(inp['bk'])
                         or np.any(inp['bv']))

    sig = inp['sig_n'].astype(np.float32)
    x = np.where(np.isfinite(sig), sig, 0.0)
    x = np.where(x == -1.0, 0.0, x).astype(np.float32)
    valid = np.any(sig != -1.0, axis=1)                # [16, 1000]

    xh = np.zeros((B, 4, L + 8), np.float32)
    xh[:, :, 4:4 + L] = x
    t1full = np.zeros((B, 36, NPAD), np.float32)
    for t in range(9):
        t1full[:, 4 * t:4 * t + 4, 0:L] = xh[:, :, t:t + L]

    meta = inp['meta'].astype(np.int64)
    e_chr = inp['emb_chr'][np.clip(meta[:, 2], 0, 22)]
    e_gene = inp['emb_gene'][np.maximum(inp['gene_id'].astype(np.int64), 0)]
    e_exon = inp['emb_exon'][np.clip(inp['exon_id'].astype(np.int64), 0, 128)]
    e_ctx = (e_chr + e_gene + e_exon).astype(np.float32)

    pos = np.arange(L, dtype=np.float32)[:, None]
    div = np.exp(np.arange(0, D, 2, dtype=np.float32) * (-np.log(10000.0) / D))
    pe = np.zeros((L, D), np.float32)
    pe[:, 0::2] = np.sin(pos * div)
    pe[:, 1::2] = np.cos(pos * div)

    ADD = np.zeros((B, D, NPAD), np.float32)
    ADD[:, :, 0] = inp['cls'][0, 0][None, :] + e_ctx
    ADD[:, :, 1:1 + L] = pe.T[None] + e_ctx[:, :, None]

    w1t = np.zeros((36, 64), np.float32)
    for t in range(9):
        w1t[4 * t:4 * t + 4] = inp['conv1_w'][:, :, t].T
    w2t = np.ascontiguousarray(inp['conv2_w'].transpose(1, 2, 0)).astype(np.float32)
    w3t = np.ascontiguousarray(inp['conv3_w'][:, :, 0].T).astype(np.float32)
    gmt = np.zeros((8, P), np.float32)
    for p in range(P):
        gmt[p // 16, p] = 1.0
    projt = np.ascontiguousarray((inp['proj'] * DN).transpose(0, 2, 1)).astype(np.float32)

    bf = lambda a: np.ascontiguousarray(np.asarray(a, np.float32).astype(ml_dtypes.bfloat16))
    shared = dict(
        w1t=w1t, w2t=w2t, w3t=w3t, gmt=gmt, projt=projt,
        wqb=bf(inp['wq']), wkb=bf(inp['wk']), wvb=bf(inp['wv']),
        wob=bf(inp['wo']),
        f1b=bf(inp['ff_w1']), f2b=bf(inp['ff_w2']),
        hw1b=bf(inp['h_w1']),
        hw2=np.ascontiguousarray(inp['h_w2'], dtype=np.float32),
    )

    in_maps = []
    for c in range(N_CORES):
        b0 = 2 * c
        real01 = np.zeros((P, 16), np.float32)
        vm01 = np.zeros((P, 16), np.float32)
        for s in range(2):
            for n in range(NPAD):
                t, row = n // P, n % P
                if n <= L:
                    real01[row, s * 8 + t] = 1.0
                    if n == 0 or valid[b0 + s, n - 1]:
                        vm01[row, s * 8 + t] = 1.0
        chw = pack_chw(inp, real01, vm01)
        in_maps.append(dict(
            shared,
            xh=np.ascontiguousarray(t1full[b0:b0 + 2]),
            addt=np.ascontiguousarray(ADD[b0:b0 + 2]),
            chw=chw,
        ))

    nc = bacc.Bacc()
    build(nc, skip_res_bias=skip_res_bias, skip_qkv_bias=skip_qkv_bias)
    nc.finalize()
    res = run_bass_kernel_spmd(nc, in_maps, list(range(N_CORES)))
    global LAST_RESULT
    LAST_RESULT = res
    out = np.concatenate([np.asarray(res.results[c]['o']).reshape(2)
                          for c in range(N_CORES)])
    return out.astype(np.float32)


LAST_RESULT = None


if __name__ == '__main__':
    import reference
    inputs = {k: np.asarray(v) for k, v in reference.setup_inputs().items()}
    got = kernel(**inputs)
    print('kernel out:', got)

